# revision 61
# baseline (speedup 1.0000x reference)
"""ArcFace logits on 8 Trainium2 NeuronCores — class-parallel (partial-FC) sharding.

Math: logits = SCALE * cos(arccos(clip(f_n @ w_n.T)) + MARGIN*onehot(targets))
Since cos(arccos(x)) == x, only the 1024 target entries need the margin
correction cos(t+m) = cos(m)*x - sin(m)*sqrt(1-x^2); everything else is just
the normalized matmul scaled by SCALE.

Device (SPMD, identical graph on all 8 cores, class-sharded; PE does ONLY the
main matmul stream):
  - shards OVERLAP: core i covers classes [i*12500, i*12500+12544) so each
    core runs 98 class-blocks (vs 100 for a disjoint 12800 split) — 2% less
    PE work; host takes [:12500] from each shard
  - startup critical path minimized: f ships as 8 per-tile DMAs on the sync +
    scalar rings, w chunk 0 goes out as 4 cb-ascending pieces on gpsimd, wn
    chunk 0 as 4 pieces on sync/scalar; the f normalize/square work is split
    ACT/DVE per tile and all 32 PE transposes precede the first matmul in the
    PE queue, so the PE starts the main stream as soon as the first w piece
    lands (~4us) with no later transpose stalls
  - w norms: ACT Square+accum_out for even c-blocks, fused DVE
    affine_mul_reduce for odd ones (classes on partitions, exactly the layout
    the evacuation scaling needs); rsqrt in quarters for chunk 0 so the first
    evacuations don't wait on the whole chunk's norms
  - f row-normalize (*SCALE folded in), cast bf16, PE-transpose -> fT
  - main matmul out[c,b] = wT.T @ fT in bf16 (fp32 PSUM); w-norm scaling fused
    into the PSUM->SBUF evacuation (per-partition scale), output cast to bf16
    (rel-err budget is 2e-2; bf16 adds ~2e-3)
  - margin deltas for all 1024 rows from gathered target weight rows, emitted
    mid-loop so they don't sit on the kernel tail
  - output DMAs per 7-cb group alternating sync/scalar rings; the LAST group
    ships per-c-block across three rings so the kernel tail is one ~260KB DMA
Host: shard/transpose/concat + apply the device-computed deltas at the 1024
target positions (pure indexing; all arithmetic happens on device).
"""

import math
import os

import numpy as np

IN_F = 512
OUT_C = 100000
B = 1024
MARGIN = 0.5
SCALE = 20.0

NCORES = 8
CSTEP = OUT_C // NCORES  # 12500 stride between shard starts
CSH = 12544            # classes per core (98 * 128), shards overlap by 44
P = 128
KT = IN_F // P         # 4 contraction subtiles
BT = B // P            # 8 batch tiles
NF = 512               # matmul moving free dim (one PSUM bank of fp32)
NB = B // NF           # 2
CW = 1792              # class chunk width streamed from DRAM (14 c-blocks)
CBK = CW // P          # 14 class blocks per chunk
CHUNKS = CSH // CW     # 7
OG = 7                 # c-blocks batched per output DMA (2 groups per chunk)

_GRAPH = None
LAST_EXEC_TIME_NS = None
LAST_RES = None


def _build_graph():
    from contextlib import ExitStack

    import concourse.bass as bass  # noqa: F401
    import concourse.tile as tile
    from concourse import bacc, mybir
    from concourse.masks import make_identity

    dt = mybir.dt
    AF = mybir.ActivationFunctionType
    ALU = mybir.AluOpType
    cosm = math.cos(MARGIN)
    sinm = math.sin(MARGIN)

    nc = bacc.Bacc()
    wT_e = nc.declare_dram_parameter("wT", [IN_F, CSH], dt.bfloat16, isOutput=False)
    wn_e = nc.declare_dram_parameter("wn", [CSH, IN_F], dt.bfloat16, isOutput=False)
    f_e = nc.declare_dram_parameter("f", [B, IN_F], dt.bfloat16, isOutput=False)
    wtg_e = nc.declare_dram_parameter("wtgt", [B, IN_F], dt.bfloat16, isOutput=False)
    out_e = nc.declare_dram_parameter("out", [CSH, B], dt.bfloat16, isOutput=True)
    dlt_e = nc.declare_dram_parameter("delta", [P, BT], dt.float32, isOutput=True)

    wT_v = wT_e[:].rearrange("(k p) c -> p k c", p=P)   # d = k*128 + p
    wn_v = wn_e[:].rearrange("(n p) d -> p n d", p=P)   # c = n*128 + p
    f_v = f_e[:].rearrange("(t p) d -> p t d", p=P)     # b = t*128 + p
    wtg_v = wtg_e[:].rearrange("(t p) d -> p t d", p=P)

    with ExitStack() as ctx:
        tc = ctx.enter_context(tile.TileContext(nc))
        cpool = ctx.enter_context(tc.tile_pool(name="cpool", bufs=1))
        fpool = ctx.enter_context(tc.tile_pool(name="fpool", bufs=1))
        # bufs=2 is deliberate: chunk ci's DMA then carries a REAL dependency
        # on chunk ci-2's last matmul (pool-buffer recycling), which is the
        # only reliable way to keep the greedy scheduler from hoisting every
        # chunk transfer into the startup window (HBM fair-shares bandwidth
        # across in-flight transfers, starving the critical first tiles)
        wpool = ctx.enter_context(tc.tile_pool(name="wpool", bufs=2))
        wnpool = ctx.enter_context(tc.tile_pool(name="wnpool", bufs=2))
        sqpool = ctx.enter_context(tc.tile_pool(name="sqpool", bufs=3))
        opool = ctx.enter_context(tc.tile_pool(name="opool", bufs=3))
        smal = ctx.enter_context(tc.tile_pool(name="smal", bufs=2))
        # one pool, 7 banks: startup transposes rotate through the same
        # buffers as the matmul accumulators (same tag), the 8th bank is
        # reserved by the framework
        po_pool = ctx.enter_context(tc.tile_pool(name="po", bufs=7, space="PSUM"))

        ident = cpool.tile([P, P], dt.bfloat16)
        make_identity(nc, ident[:])

        # pre-warm the ACT table sets (Sqrt first — it gates rnf20 on the
        # critical path — then Square) during the initial DMA window;
        # otherwise the ~1.5us ACT_TABLE_LOADs land at first real use
        twarm = cpool.tile([P, 1], dt.float32)
        nc.gpsimd.memset(twarm[:], 1.0)
        twout = cpool.tile([P, 1], dt.float32)
        nc.scalar.activation(twout[:], twarm[:], AF.Sqrt, scale=1.0)
        nc.scalar.activation(twout[:], twarm[:], AF.Square)

        # ---------------- DMA priority via per-queue FIFO --------------------
        # Aggregate HBM bandwidth (~358GB/s/core) fair-shares across all
        # queued transfers, so anything queued early delays everything else.
        # Strict ordering: f tiles lead on the sync+scalar rings; the wT
        # stream lives entirely on the gpsimd SWDGE queue in cb-ascending
        # order; wn lives on sync BEHIND f; outputs ride scalar later.
        # A single DMA queue moves only ~25-34GB/s; aggregate bandwidth needs
        # MANY CONCURRENT transfers. First wave: the 8 f tiles fan out across
        # the sync+scalar queue sets (one queue each), wn0's first block gets
        # its own small slot, and the first two w c-blocks ride gpsimd SWDGE
        # queues — everything else is gated behind the f chain.
        f_sb = fpool.tile([P, BT, IN_F], dt.bfloat16)
        w_sb0 = wpool.tile([P, KT, CW], dt.bfloat16, tag="wchunk", name="w_sb0")
        wn_sb0 = wnpool.tile([P, CBK, IN_F], dt.bfloat16, tag="wnchunk", name="wn_sb0")
        for t in (0, 2, 4, 6):
            nc.sync.dma_start(f_sb[:, t], f_v[:, t])
        for t in (1, 3, 5, 7):
            nc.scalar.dma_start(f_sb[:, t], f_v[:, t])
        nc.gpsimd.dma_start(w_sb0[:, :, 0:P], wT_v[:, :, 0:P])
        nc.gpsimd.dma_start(wn_sb0[:, 0:1], wn_v[:, 0:1])
        nc.gpsimd.dma_start(w_sb0[:, :, P : 2 * P], wT_v[:, :, P : 2 * P])
        nc.gpsimd.dma_start(wn_sb0[:, 1:4], wn_v[:, 1:4])

        # ---------------- f normalize + transpose, pipelined per tile -------
        # First half all on DVE (affine_mul_reduce needs no ACT table; the
        # two ACT table loads take ~2.6us and would gate the chain).
        nf2 = smal.tile([P, BT], dt.float32)
        rnf20 = smal.tile([P, BT], dt.float32)
        f_n = fpool.tile([P, BT, IN_F], dt.bfloat16)
        fT = fpool.tile([P, KT, B], dt.bfloat16)

        def emit_f_tile(t):
            # fully per-tile pipeline: square -> recip -> sqrt -> normalize
            # -> 4 transposes -> fT copy. No cross-tile barrier — the old
            # half-granular reciprocal waited on ALL four squares, a
            # dependency the greedy scheduler could not break.
            sq = sqpool.tile([P, IN_F], dt.bfloat16, tag="sqscratch")
            if t % 2 == 0:
                nc.vector.affine_mul_reduce(
                    sq[:], nf2[:, t : t + 1], f_sb[:, t], f_sb[:, t], 1.0, 0.0
                )
            else:
                nc.scalar.activation(
                    sq[:], f_sb[:, t], AF.Square, accum_out=nf2[:, t : t + 1]
                )
            rec = smal.tile([P, 1], dt.float32, tag="recf")
            nc.vector.reciprocal(rec[:], nf2[:, t : t + 1])
            # sqrt(SCALE^2 / nf2) = SCALE * rsqrt(nf2)
            nc.scalar.activation(
                rnf20[:, t : t + 1], rec[:], AF.Sqrt, scale=SCALE * SCALE
            )
            nc.vector.tensor_scalar_mul(f_n[:, t], f_sb[:, t], rnf20[:, t : t + 1])

        def emit_f_transpose(t):
            ps = po_pool.tile([P, KT, P], dt.bfloat16, tag="pso")
            for k in range(KT):
                nc.tensor.transpose(
                    ps[:, k], f_n[:, t, k * P : (k + 1) * P], ident[:]
                )
            if t % 2 == 0:
                nc.vector.tensor_copy(fT[:, :, t * P : (t + 1) * P], ps[:])
            else:
                nc.scalar.copy(fT[:, :, t * P : (t + 1) * P], ps[:])

        for t in range(BT):
            emit_f_tile(t)
        for t in range(4):
            # t4-7 transposes are deferred into the og0-special loop: their
            # f chains drain behind chunk-0 norm work, and an early PE-queue
            # slot would stall the matmul stream on them
            emit_f_transpose(t)

        def gate(dst, src):
            # tiny gpsimd write into a DMA's destination region: the DMA
            # inherits a write-after-write dependency, i.e. a REAL "not
            # before src is ready" gate the scheduler can't hoist past;
            # gpsimd so the gate never head-of-line-blocks ACT/DVE.
            # ONLY safe on a fresh (never-written) tile — a mid-tile write
            # after earlier DMAs makes the framework duplicate the DMA.
            nc.gpsimd.tensor_copy(dst, src)

        gscr = cpool.tile([1, 1], dt.float32)

        def gate_r(region, src):
            # WAR variant for partially-written tiles: READ the destination
            # region (+ the gate source) so the DMA waits on the reader.
            # DVE (gpsimd can't codegen a generic TensorTensor); the wait is
            # on rnf20 which the DVE queue has just produced, so no HOL risk
            nc.vector.tensor_tensor(gscr[:], src, region, op=ALU.bypass)

        # chunk-0 bulk rides ungated right behind cb0-1 on the gpsimd queue;
        # wn0's tail is gated on the first-half f squares so the earliest
        # window carries only ~2MB of critical transfers
        nc.gpsimd.dma_start(w_sb0[:, :, 2 * P : 7 * P], wT_v[:, :, 2 * P : 7 * P])
        nc.gpsimd.dma_start(w_sb0[:, :, 7 * P : CW], wT_v[:, :, 7 * P : CW])
        gate_r(wn_sb0[0:1, 4:5, 0:1], nf2[0:1, 3:4])
        nc.sync.dma_start(wn_sb0[:, 4:CBK], wn_v[:, 4:CBK])

        # ---------------- w chunk streams (steady state) --------------------
        # chunk ci's transfers are gated on chunk ci-2's first output-group
        # tile (written mid-ci-2), giving each chunk ~1.5 windows of
        # transfer time without polluting the startup burst
        def emit_load(ci, gsrc):
            w_sb = wpool.tile([P, KT, CW], dt.bfloat16, tag="wchunk", name="w_sb")
            wn_sb = wnpool.tile([P, CBK, IN_F], dt.bfloat16, tag="wnchunk", name="wn_sb")
            half = (CBK // 2) * P
            gate(w_sb[0:1, 0:1, 0:1], gsrc)
            nc.gpsimd.dma_start(
                w_sb[:, :, :half], wT_v[:, :, ci * CW : ci * CW + half]
            )
            gate(w_sb[0:1, 0:1, half : half + 1], gsrc)
            nc.gpsimd.dma_start(
                w_sb[:, :, half:], wT_v[:, :, ci * CW + half : (ci + 1) * CW]
            )
            gate(wn_sb[0:1, 0:1, 0:1], gsrc)
            nc.sync.dma_start(wn_sb[:], wn_v[:, ci * CBK : (ci + 1) * CBK])
            return w_sb, wn_sb

        def emit_norm(wn_sb, first=False):
            """Per-class 1/||w|| for one chunk -> [128, CBK], classes on
            partitions. ACT Square+accum for even blocks, fused DVE
            affine_mul_reduce for odd; rsqrt in quarters (chunk 0) or halves
            so early evacuations don't wait on the whole chunk's norms."""
            rnw = smal.tile([P, CBK], dt.float32, tag="rnw", name="rnw")
            nw2 = smal.tile([P, CBK], dt.float32, tag="nw2", name="nw2")
            bounds = [4, 8, 11, CBK] if first else [CBK // 2, CBK]
            done = 0
            for g in range(CBK):
                norm_block(wn_sb, nw2, g)
                if g + 1 in bounds:
                    norm_rsqrt(nw2, rnw, done, g + 1)
                    done = g + 1
            return rnw

        def norm_block(wn_sb, nw2, g, act=None):
            sq = sqpool.tile([P, IN_F], dt.bfloat16, tag="sqscratch")
            if (g % 2 == 0) if act is None else act:
                nc.scalar.activation(
                    sq[:], wn_sb[:, g], AF.Square, accum_out=nw2[:, g : g + 1]
                )
            else:
                nc.vector.affine_mul_reduce(
                    sq[:], nw2[:, g : g + 1], wn_sb[:, g], wn_sb[:, g], 1.0, 0.0
                )

        def norm_rsqrt(nw2, rnw, a, b):
            recw = smal.tile([P, b - a], dt.float32, tag=f"recw{a}", name="recw")
            nc.vector.reciprocal(recw[:], nw2[:, a:b])
            nc.scalar.activation(rnw[:, a:b], recw[:], AF.Sqrt, scale=1.0)

        # margin input DMA deferred to mid-loop
        wt_sb = fpool.tile([P, BT, IN_F], dt.bfloat16, name="wt_sb")
        nt2 = smal.tile([P, BT], dt.float32, name="nt2")
        drot = smal.tile([P, BT], dt.float32, name="drot")

        def emit_margin_dot(t):
            sq = sqpool.tile([P, IN_F], dt.bfloat16, tag="sqscratch")
            nc.scalar.activation(
                sq[:], wt_sb[:, t], AF.Square, accum_out=nt2[:, t : t + 1]
            )
            prod = sqpool.tile([P, IN_F], dt.bfloat16, tag="sqscratch")
            nc.vector.affine_mul_reduce(
                prod[:], drot[:, t : t + 1], f_sb[:, t], wt_sb[:, t], 1.0, 0.0
            )

        def emit_margin():
            rec_t = smal.tile([P, BT], dt.float32)
            nc.vector.reciprocal(rec_t[:], nt2[:])
            rnt = smal.tile([P, BT], dt.float32)
            nc.scalar.activation(rnt[:], rec_t[:], AF.Sqrt, scale=1.0)
            u = smal.tile([P, BT], dt.float32)
            nc.vector.tensor_mul(u[:], drot[:], rnf20[:])
            nc.vector.tensor_mul(u[:], u[:], rnt[:])          # u = SCALE * cos_t
            t1 = smal.tile([P, BT], dt.float32)
            nc.vector.tensor_mul(t1[:], u[:], u[:])
            nc.vector.tensor_scalar(t1[:], t1[:], -1.0, SCALE * SCALE, ALU.mult, ALU.add)
            nc.vector.tensor_scalar_max(t1[:], t1[:], 0.0)    # max(S^2 - u^2, 0)
            s_t = smal.tile([P, BT], dt.float32)
            nc.scalar.activation(s_t[:], t1[:], AF.Sqrt, scale=1.0)  # SCALE*sin_t
            t2 = smal.tile([P, BT], dt.float32)
            nc.vector.tensor_scalar_mul(t2[:], s_t[:], -sinm)
            t3 = smal.tile([P, BT], dt.float32)
            nc.vector.tensor_scalar_mul(t3[:], u[:], cosm - 1.0)
            delta = smal.tile([P, BT], dt.float32)
            nc.vector.tensor_add(delta[:], t2[:], t3[:])
            nc.sync.dma_start(dlt_e[:], delta[:])

        # ---------------- main class loop ------------------------------------
        def emit_mms(w_sb, cb, nb):
            pso = po_pool.tile([P, NF], dt.float32, tag="pso", name=f"pso{nb}")
            for k in range(KT):
                nc.tensor.matmul(
                    pso[:],
                    lhsT=w_sb[:, k, cb * P : (cb + 1) * P],
                    rhs=fT[:, k, nb * NF : (nb + 1) * NF],
                    start=(k == 0),
                    stop=(k == KT - 1),
                )
            return pso

        def emit_evac(pso, rnw, cb, nb, osb, cbi, eidx, act_major=True,
                      force_act=False):
            # og1 carries the next chunk's norm blocks, so tilt the
            # evacuation split toward DVE there (ACT holds the squares) —
            # EXCEPT its last c-blocks, whose banks the next og's first
            # matmuls recycle: those go ACT unconditionally so a DVE
            # backlog can't stall the PE across the og boundary
            if force_act or ((eidx % 5 < 3) if act_major else (eidx % 5 >= 2)):
                nc.scalar.activation(
                    osb[:, cbi, nb * NF : (nb + 1) * NF],
                    pso[:], AF.Copy,
                    scale=rnw[:, cb : cb + 1],
                )
            else:
                nc.vector.tensor_scalar_mul(
                    osb[:, cbi, nb * NF : (nb + 1) * NF],
                    pso[:], rnw[:, cb : cb + 1],
                )

        ready = {}   # ci -> (w_sb, rnw)
        raw = {}     # ci -> (w_sb, wn_sb)
        # chunk-0 norms: blocks 0-6 up front (og0's evacuations need them),
        # blocks 7-13 woven into the og0 nb1 pass so they can't queue on ACT
        # ahead of the nb1 evacuations; rsqrt in thirds
        rnw0 = smal.tile([P, CBK], dt.float32, tag="rnw", name="rnw0")
        nw20 = smal.tile([P, CBK], dt.float32, tag="nw2", name="nw20")
        for g in range(7):
            norm_block(wn_sb0, nw20, g)
            if g == 3:
                norm_rsqrt(nw20, rnw0, 0, 4)
        norm_rsqrt(nw20, rnw0, 4, 7)
        ready[0] = (w_sb0, rnw0)
        raw[1] = emit_load(1, rnf20[0:1, 6:7])
        osb_og0 = None  # og0 output tile of the current chunk (prefetch gate)

        for ci in range(CHUNKS):
            w_sb, rnw = ready.pop(ci)
            # norms for the NEXT chunk are spread over this chunk's two
            # output groups: ACT Square blocks (g 0-6) in og0's late slots,
            # DVE affine_mul_reduce blocks (g 7-13) in og1's early slots —
            # never a solid norm burst that starves evacuations and stalls
            # the PE on PSUM-bank recycling. Chunk 1's wn arrives mid-og1,
            # so its norms are emitted in bulk after chunk 0's og1 evacs.
            nxt = None
            if ci + 1 in raw and ci > 0:
                nw_sb, nwn_sb = raw.pop(ci + 1)
                nrnw = smal.tile([P, CBK], dt.float32, tag="rnw", name="rnw")
                nnw2 = smal.tile([P, CBK], dt.float32, tag="nw2", name="nw2")
                nxt = (nw_sb, nwn_sb, nrnw, nnw2)
            for og in range(CBK // OG):
                if og == 0 and ci == 2:
                    # margin input: gated on this chunk's rnw (~mid-kernel)
                    # so the greedy scheduler can't hoist the 1MB transfer
                    # into the startup window
                    gate(wt_sb[0:1, 0:1, 0:1], rnw[0:1, 0:1])
                    nc.gpsimd.dma_start(wt_sb[:], wtg_v)
                if og == 1 and ci == 0 and ci + 1 in raw:
                    nw_sb, nwn_sb = raw.pop(ci + 1)
                    nrnw = smal.tile([P, CBK], dt.float32, tag="rnw", name="rnw")
                    nnw2 = smal.tile([P, CBK], dt.float32, tag="nw2", name="nw2")
                    nxt = (nw_sb, nwn_sb, nrnw, nnw2)
                osb = opool.tile([P, OG, B], dt.bfloat16, tag="osb")
                if ci == 0 and og == 0:
                    # chunk-0 special: all nb=0 matmuls first (they only need
                    # the earlier fT tiles), then the nb=1 pass — the second
                    # batch half's transposes (emitted per-tile above) have
                    # ~12us of nb0 work to complete behind
                    for cbi in range(OG):
                        pso = emit_mms(w_sb, cbi, 0)
                        emit_evac(pso, rnw, cbi, 0, osb, cbi, cbi * NB)
                        if 3 <= cbi < 7:
                            emit_f_transpose(cbi + 1)
                    for cbi in range(OG):
                        pso = emit_mms(w_sb, cbi, 1)
                        emit_evac(pso, rnw, cbi, 1, osb, cbi, cbi * NB + 1)
                        norm_block(wn_sb0, nw20, 7 + cbi)
                    norm_rsqrt(nw20, rnw0, 7, CBK)
                else:
                    for cbi in range(OG):
                        cb = og * OG + cbi
                        pso_pair = [emit_mms(w_sb, cb, nb) for nb in range(NB)]
                        for nb in range(NB):
                            # og1 psos #7 (cbi3-nb1) and #8 (cbi4-nb0) are
                            # exactly the banks the next og0's first matmuls
                            # recycle (allocation phase is constant: 36+28ci
                            # = 1 mod 7) — force those two onto ACT so a DVE
                            # backlog can't stall the PE across the boundary
                            emit_evac(
                                pso_pair[nb], rnw, cb, nb, osb, cbi,
                                (ci * CBK + cb) * NB + nb,
                                act_major=(og == 0 or cbi >= 4),
                                force_act=(og == 1 and (
                                    (cbi == 3 and nb == 1)
                                    or (cbi == 4 and nb == 0))),
                            )
                        if nxt is not None and ci > 0 and og == 1:
                            norm_block(nxt[1], nxt[3], 2 * cbi)
                            norm_block(nxt[1], nxt[3], 2 * cbi + 1)
                            if cbi == 3:
                                norm_rsqrt(nxt[3], nxt[2], 0, 7)
                        if og == 0 and ci == 3:
                            emit_margin_dot(cbi)
                        if og == 0 and ci == 4 and cbi == 0:
                            emit_margin_dot(7)
                        if og == 0 and ci == 4 and cbi == 1:
                            emit_margin()
                if og == 1 and nxt is not None:
                    if ci == 0:
                        # bulk after evacs (see above)
                        for g in range(CBK):
                            norm_block(nxt[1], nxt[3], g)
                            if g == 6:
                                norm_rsqrt(nxt[3], nxt[2], 0, 7)
                    norm_rsqrt(nxt[3], nxt[2], 7, CBK)
                    ready[ci + 1] = (nxt[0], nxt[2])
                if og == 0:
                    osb_og0 = osb
                if og == 1 and ci + 2 < CHUNKS:
                    raw[ci + 2] = emit_load(ci + 2, osb_og0[0:1, 0:1, 0:1])
                row0 = ci * CW + og * OG * P
                last_group = ci == CHUNKS - 1 and og == (CBK // OG) - 1
                if last_group:
                    # tail killer: ship the last group per-c-block across
                    # three rings so the kernel ends on a ~260KB transfer
                    engs = [nc.sync, nc.scalar, nc.gpsimd]
                    for cbi in range(OG):
                        engs[cbi % 3].dma_start(
                            out_e[row0 + cbi * P : row0 + (cbi + 1) * P, :]
                            .rearrange("(g p) b -> p g b", p=P),
                            osb[:, cbi : cbi + 1],
                        )
                else:
                    nc.scalar.dma_start(
                        out_e[row0 : row0 + OG * P, :].rearrange(
                            "(g p) b -> p g b", p=P
                        ),
                        osb[:],
                    )

    nc.finalize()
    return nc


def _prep_inputs(features, targets, weights):
    import ml_dtypes

    f32 = np.ascontiguousarray(np.asarray(features, dtype=np.float32))
    tgt = np.asarray(targets).astype(np.int64)
    w = np.asarray(weights, dtype=np.float32)

    wide = (NCORES - 1) * CSTEP + CSH  # 100044
    wpad = np.zeros((wide, IN_F), dtype=np.float32)
    wpad[:OUT_C] = w
    wpad[OUT_C:, 0] = 1.0  # unit-norm filler rows: no inf/nan anywhere

    fb = f32.astype(ml_dtypes.bfloat16)
    in_maps = []
    for i in range(NCORES):
        sh = wpad[i * CSTEP : i * CSTEP + CSH]
        shb = sh.astype(ml_dtypes.bfloat16)
        wT = np.ascontiguousarray(shb.T)
        loc = np.clip(tgt - i * CSTEP, 0, CSH - 1)
        wtgt = np.ascontiguousarray(shb[loc])
        in_maps.append({"wT": wT, "wn": shb, "f": fb, "wtgt": wtgt})
    return in_maps, tgt


def kernel(features, targets, weights):
    global _GRAPH, LAST_EXEC_TIME_NS, LAST_RES
    from concourse.bass_utils import run_bass_kernel_spmd

    if _GRAPH is None:
        _GRAPH = _build_graph()
    nc = _GRAPH

    in_maps, tgt = _prep_inputs(features, targets, weights)

    trace = bool(int(os.environ.get("BASS_KERNEL_TRACE", "0")))
    res = run_bass_kernel_spmd(nc, in_maps, core_ids=list(range(NCORES)), trace=trace)
    LAST_EXEC_TIME_NS = res.exec_time_ns
    LAST_RES = res

    outs = [res.results[i]["out"][:CSTEP] for i in range(NCORES)]  # [12500, B]
    full = np.concatenate(outs, axis=0)                 # [100000, B] bf16
    logits = np.ascontiguousarray(full.T, dtype=np.float32)     # [B, OUT_C] f32

    # apply device-computed margin deltas at the 1024 target positions
    deltas = np.stack(
        [res.results[i]["delta"].T.reshape(B) for i in range(NCORES)]
    )  # [NCORES, B]; delta[p, t] -> b = t*128 + p
    rows = np.arange(B)
    core_of = np.minimum(tgt // CSTEP, NCORES - 1).astype(np.int64)
    logits[rows, tgt] += deltas[core_of, rows]
    return logits


# revision 62
# speedup vs baseline: 1.0077x; 1.0077x over previous
"""ArcFace logits on 8 Trainium2 NeuronCores — class-parallel (partial-FC) sharding.

Math: logits = SCALE * cos(arccos(clip(f_n @ w_n.T)) + MARGIN*onehot(targets))
Since cos(arccos(x)) == x, only the 1024 target entries need the margin
correction cos(t+m) = cos(m)*x - sin(m)*sqrt(1-x^2); everything else is just
the normalized matmul scaled by SCALE.

Device (SPMD, identical graph on all 8 cores, class-sharded; PE does ONLY the
main matmul stream):
  - shards OVERLAP: core i covers classes [i*12500, i*12500+12544) so each
    core runs 98 class-blocks (vs 100 for a disjoint 12800 split) — 2% less
    PE work; host takes [:12500] from each shard
  - startup critical path minimized: f ships as 8 per-tile DMAs on the sync +
    scalar rings, w chunk 0 goes out as 4 cb-ascending pieces on gpsimd, wn
    chunk 0 as 4 pieces on sync/scalar; the f normalize/square work is split
    ACT/DVE per tile and all 32 PE transposes precede the first matmul in the
    PE queue, so the PE starts the main stream as soon as the first w piece
    lands (~4us) with no later transpose stalls
  - w norms: ACT Square+accum_out for even c-blocks, fused DVE
    affine_mul_reduce for odd ones (classes on partitions, exactly the layout
    the evacuation scaling needs); rsqrt in quarters for chunk 0 so the first
    evacuations don't wait on the whole chunk's norms
  - f row-normalize (*SCALE folded in), cast bf16, PE-transpose -> fT
  - main matmul out[c,b] = wT.T @ fT in bf16 (fp32 PSUM); w-norm scaling fused
    into the PSUM->SBUF evacuation (per-partition scale), output cast to bf16
    (rel-err budget is 2e-2; bf16 adds ~2e-3)
  - margin deltas for all 1024 rows from gathered target weight rows, emitted
    mid-loop so they don't sit on the kernel tail
  - output DMAs per 7-cb group alternating sync/scalar rings; the LAST group
    ships per-c-block across three rings so the kernel tail is one ~260KB DMA
Host: shard/transpose/concat + apply the device-computed deltas at the 1024
target positions (pure indexing; all arithmetic happens on device).
"""

import math
import os

import numpy as np

IN_F = 512
OUT_C = 100000
B = 1024
MARGIN = 0.5
SCALE = 20.0

NCORES = 8
CSTEP = OUT_C // NCORES  # 12500 stride between shard starts
CSH = 12544            # classes per core (98 * 128), shards overlap by 44
P = 128
KT = IN_F // P         # 4 contraction subtiles
BT = B // P            # 8 batch tiles
NF = 512               # matmul moving free dim (one PSUM bank of fp32)
NB = B // NF           # 2
CW = 1792              # class chunk width streamed from DRAM (14 c-blocks)
CBK = CW // P          # 14 class blocks per chunk
CHUNKS = CSH // CW     # 7
OG = 7                 # c-blocks batched per output DMA (2 groups per chunk)

_GRAPH = None
LAST_EXEC_TIME_NS = None
LAST_RES = None


def _build_graph():
    from contextlib import ExitStack

    import concourse.bass as bass  # noqa: F401
    import concourse.tile as tile
    from concourse import bacc, mybir
    from concourse.masks import make_identity

    dt = mybir.dt
    AF = mybir.ActivationFunctionType
    ALU = mybir.AluOpType
    cosm = math.cos(MARGIN)
    sinm = math.sin(MARGIN)

    nc = bacc.Bacc()
    wT_e = nc.declare_dram_parameter("wT", [IN_F, CSH], dt.bfloat16, isOutput=False)
    wn_e = nc.declare_dram_parameter("wn", [CSH, IN_F], dt.bfloat16, isOutput=False)
    f_e = nc.declare_dram_parameter("f", [B, IN_F], dt.bfloat16, isOutput=False)
    wtg_e = nc.declare_dram_parameter("wtgt", [B, IN_F], dt.bfloat16, isOutput=False)
    out_e = nc.declare_dram_parameter("out", [CSH, B], dt.bfloat16, isOutput=True)
    dlt_e = nc.declare_dram_parameter("delta", [P, BT], dt.float32, isOutput=True)

    wT_v = wT_e[:].rearrange("(k p) c -> p k c", p=P)   # d = k*128 + p
    wn_v = wn_e[:].rearrange("(n p) d -> p n d", p=P)   # c = n*128 + p
    f_v = f_e[:].rearrange("(t p) d -> p t d", p=P)     # b = t*128 + p
    wtg_v = wtg_e[:].rearrange("(t p) d -> p t d", p=P)

    with ExitStack() as ctx:
        tc = ctx.enter_context(tile.TileContext(nc))
        cpool = ctx.enter_context(tc.tile_pool(name="cpool", bufs=1))
        fpool = ctx.enter_context(tc.tile_pool(name="fpool", bufs=1))
        # bufs=2 is deliberate: chunk ci's DMA then carries a REAL dependency
        # on chunk ci-2's last matmul (pool-buffer recycling), which is the
        # only reliable way to keep the greedy scheduler from hoisting every
        # chunk transfer into the startup window (HBM fair-shares bandwidth
        # across in-flight transfers, starving the critical first tiles)
        wpool = ctx.enter_context(tc.tile_pool(name="wpool", bufs=2))
        wnpool = ctx.enter_context(tc.tile_pool(name="wnpool", bufs=2))
        sqpool = ctx.enter_context(tc.tile_pool(name="sqpool", bufs=3))
        opool = ctx.enter_context(tc.tile_pool(name="opool", bufs=3))
        smal = ctx.enter_context(tc.tile_pool(name="smal", bufs=2))
        # one pool, 7 banks: startup transposes rotate through the same
        # buffers as the matmul accumulators (same tag), the 8th bank is
        # reserved by the framework
        po_pool = ctx.enter_context(tc.tile_pool(name="po", bufs=7, space="PSUM"))

        ident = cpool.tile([P, P], dt.bfloat16)
        make_identity(nc, ident[:])

        # pre-warm the ACT table sets (Sqrt first — it gates rnf20 on the
        # critical path — then Square) during the initial DMA window;
        # otherwise the ~1.5us ACT_TABLE_LOADs land at first real use
        twarm = cpool.tile([P, 1], dt.float32)
        nc.gpsimd.memset(twarm[:], 1.0)
        twout = cpool.tile([P, 1], dt.float32)
        nc.scalar.activation(twout[:], twarm[:], AF.Sqrt, scale=1.0)
        nc.scalar.activation(twout[:], twarm[:], AF.Square)

        # ---------------- DMA priority via per-queue FIFO --------------------
        # Aggregate HBM bandwidth (~358GB/s/core) fair-shares across all
        # queued transfers, so anything queued early delays everything else.
        # Strict ordering: f tiles lead on the sync+scalar rings; the wT
        # stream lives entirely on the gpsimd SWDGE queue in cb-ascending
        # order; wn lives on sync BEHIND f; outputs ride scalar later.
        # A single DMA queue moves only ~25-34GB/s; aggregate bandwidth needs
        # MANY CONCURRENT transfers. First wave: the 8 f tiles fan out across
        # the sync+scalar queue sets (one queue each), wn0's first block gets
        # its own small slot, and the first two w c-blocks ride gpsimd SWDGE
        # queues — everything else is gated behind the f chain.
        f_sb = fpool.tile([P, BT, IN_F], dt.bfloat16)
        w_sb0 = wpool.tile([P, KT, CW], dt.bfloat16, tag="wchunk", name="w_sb0")
        wn_sb0 = wnpool.tile([P, CBK, IN_F], dt.bfloat16, tag="wnchunk", name="wn_sb0")
        for t in (0, 2, 4, 6):
            nc.sync.dma_start(f_sb[:, t], f_v[:, t])
        for t in (1, 3, 5, 7):
            nc.scalar.dma_start(f_sb[:, t], f_v[:, t])
        nc.gpsimd.dma_start(w_sb0[:, :, 0:P], wT_v[:, :, 0:P])
        nc.gpsimd.dma_start(wn_sb0[:, 0:1], wn_v[:, 0:1])
        nc.gpsimd.dma_start(w_sb0[:, :, P : 2 * P], wT_v[:, :, P : 2 * P])
        nc.gpsimd.dma_start(wn_sb0[:, 1:4], wn_v[:, 1:4])

        # ---------------- f normalize + transpose, pipelined per tile -------
        # First half all on DVE (affine_mul_reduce needs no ACT table; the
        # two ACT table loads take ~2.6us and would gate the chain).
        nf2 = smal.tile([P, BT], dt.float32)
        rnf20 = smal.tile([P, BT], dt.float32)
        f_n = fpool.tile([P, BT, IN_F], dt.bfloat16)
        fT = fpool.tile([P, KT, B], dt.bfloat16)

        def emit_f_tile(t):
            # fully per-tile pipeline: square -> recip -> sqrt -> normalize
            # -> 4 transposes -> fT copy. No cross-tile barrier — the old
            # half-granular reciprocal waited on ALL four squares, a
            # dependency the greedy scheduler could not break.
            sq = sqpool.tile([P, IN_F], dt.bfloat16, tag="sqscratch")
            if t % 2 == 0:
                nc.vector.affine_mul_reduce(
                    sq[:], nf2[:, t : t + 1], f_sb[:, t], f_sb[:, t], 1.0, 0.0
                )
            else:
                nc.scalar.activation(
                    sq[:], f_sb[:, t], AF.Square, accum_out=nf2[:, t : t + 1]
                )
            rec = smal.tile([P, 1], dt.float32, tag="recf")
            nc.vector.reciprocal(rec[:], nf2[:, t : t + 1])
            # sqrt(SCALE^2 / nf2) = SCALE * rsqrt(nf2)
            nc.scalar.activation(
                rnf20[:, t : t + 1], rec[:], AF.Sqrt, scale=SCALE * SCALE
            )
            nc.vector.tensor_scalar_mul(f_n[:, t], f_sb[:, t], rnf20[:, t : t + 1])

        def emit_f_transpose(t):
            ps = po_pool.tile([P, KT, P], dt.bfloat16, tag="pso")
            for k in range(KT):
                nc.tensor.transpose(
                    ps[:, k], f_n[:, t, k * P : (k + 1) * P], ident[:]
                )
            if t % 2 == 0:
                nc.vector.tensor_copy(fT[:, :, t * P : (t + 1) * P], ps[:])
            else:
                nc.scalar.copy(fT[:, :, t * P : (t + 1) * P], ps[:])

        for t in range(BT):
            emit_f_tile(t)
        for t in range(4):
            # t4-7 transposes are deferred into the og0-special loop: their
            # f chains drain behind chunk-0 norm work, and an early PE-queue
            # slot would stall the matmul stream on them
            emit_f_transpose(t)

        def gate(dst, src):
            # tiny gpsimd write into a DMA's destination region: the DMA
            # inherits a write-after-write dependency, i.e. a REAL "not
            # before src is ready" gate the scheduler can't hoist past;
            # gpsimd so the gate never head-of-line-blocks ACT/DVE.
            # ONLY safe on a fresh (never-written) tile — a mid-tile write
            # after earlier DMAs makes the framework duplicate the DMA.
            nc.gpsimd.tensor_copy(dst, src)

        gscr = cpool.tile([1, 1], dt.float32)

        def gate_r(region, src):
            # WAR variant for partially-written tiles: READ the destination
            # region (+ the gate source) so the DMA waits on the reader.
            # DVE (gpsimd can't codegen a generic TensorTensor); the wait is
            # on rnf20 which the DVE queue has just produced, so no HOL risk
            nc.vector.tensor_tensor(gscr[:], src, region, op=ALU.bypass)

        # chunk-0 bulk rides ungated right behind cb0-1 on the gpsimd queue;
        # wn0's tail is gated on the first-half f squares so the earliest
        # window carries only ~2MB of critical transfers
        nc.gpsimd.dma_start(w_sb0[:, :, 2 * P : 7 * P], wT_v[:, :, 2 * P : 7 * P])
        nc.gpsimd.dma_start(w_sb0[:, :, 7 * P : CW], wT_v[:, :, 7 * P : CW])
        gate_r(wn_sb0[0:1, 4:5, 0:1], nf2[0:1, 3:4])
        nc.sync.dma_start(wn_sb0[:, 4:CBK], wn_v[:, 4:CBK])

        # ---------------- w chunk streams (steady state) --------------------
        # chunk ci's transfers are gated on chunk ci-2's first output-group
        # tile (written mid-ci-2), giving each chunk ~1.5 windows of
        # transfer time without polluting the startup burst
        def emit_load(ci, gsrc):
            w_sb = wpool.tile([P, KT, CW], dt.bfloat16, tag="wchunk", name="w_sb")
            wn_sb = wnpool.tile([P, CBK, IN_F], dt.bfloat16, tag="wnchunk", name="wn_sb")
            half = (CBK // 2) * P
            gate(w_sb[0:1, 0:1, 0:1], gsrc)
            nc.gpsimd.dma_start(
                w_sb[:, :, :half], wT_v[:, :, ci * CW : ci * CW + half]
            )
            gate(w_sb[0:1, 0:1, half : half + 1], gsrc)
            nc.gpsimd.dma_start(
                w_sb[:, :, half:], wT_v[:, :, ci * CW + half : (ci + 1) * CW]
            )
            gate(wn_sb[0:1, 0:1, 0:1], gsrc)
            nc.sync.dma_start(wn_sb[:], wn_v[:, ci * CBK : (ci + 1) * CBK])
            return w_sb, wn_sb

        def emit_norm(wn_sb, first=False):
            """Per-class 1/||w|| for one chunk -> [128, CBK], classes on
            partitions. ACT Square+accum for even blocks, fused DVE
            affine_mul_reduce for odd; rsqrt in quarters (chunk 0) or halves
            so early evacuations don't wait on the whole chunk's norms."""
            rnw = smal.tile([P, CBK], dt.float32, tag="rnw", name="rnw")
            nw2 = smal.tile([P, CBK], dt.float32, tag="nw2", name="nw2")
            bounds = [4, 8, 11, CBK] if first else [CBK // 2, CBK]
            done = 0
            for g in range(CBK):
                norm_block(wn_sb, nw2, g)
                if g + 1 in bounds:
                    norm_rsqrt(nw2, rnw, done, g + 1)
                    done = g + 1
            return rnw

        def norm_block(wn_sb, nw2, g, act=None):
            sq = sqpool.tile([P, IN_F], dt.bfloat16, tag="sqscratch")
            if (g % 2 == 0) if act is None else act:
                nc.scalar.activation(
                    sq[:], wn_sb[:, g], AF.Square, accum_out=nw2[:, g : g + 1]
                )
            else:
                nc.vector.affine_mul_reduce(
                    sq[:], nw2[:, g : g + 1], wn_sb[:, g], wn_sb[:, g], 1.0, 0.0
                )

        def norm_rsqrt(nw2, rnw, a, b):
            recw = smal.tile([P, b - a], dt.float32, tag=f"recw{a}", name="recw")
            nc.vector.reciprocal(recw[:], nw2[:, a:b])
            nc.scalar.activation(rnw[:, a:b], recw[:], AF.Sqrt, scale=1.0)

        # margin input DMA deferred to mid-loop
        wt_sb = fpool.tile([P, BT, IN_F], dt.bfloat16, name="wt_sb")
        nt2 = smal.tile([P, BT], dt.float32, name="nt2")
        drot = smal.tile([P, BT], dt.float32, name="drot")

        def emit_margin_dot(t):
            sq = sqpool.tile([P, IN_F], dt.bfloat16, tag="sqscratch")
            nc.scalar.activation(
                sq[:], wt_sb[:, t], AF.Square, accum_out=nt2[:, t : t + 1]
            )
            prod = sqpool.tile([P, IN_F], dt.bfloat16, tag="sqscratch")
            nc.vector.affine_mul_reduce(
                prod[:], drot[:, t : t + 1], f_sb[:, t], wt_sb[:, t], 1.0, 0.0
            )

        def emit_margin():
            rec_t = smal.tile([P, BT], dt.float32)
            nc.vector.reciprocal(rec_t[:], nt2[:])
            rnt = smal.tile([P, BT], dt.float32)
            nc.scalar.activation(rnt[:], rec_t[:], AF.Sqrt, scale=1.0)
            u = smal.tile([P, BT], dt.float32)
            nc.vector.tensor_mul(u[:], drot[:], rnf20[:])
            nc.vector.tensor_mul(u[:], u[:], rnt[:])          # u = SCALE * cos_t
            t1 = smal.tile([P, BT], dt.float32)
            nc.vector.tensor_mul(t1[:], u[:], u[:])
            nc.vector.tensor_scalar(t1[:], t1[:], -1.0, SCALE * SCALE, ALU.mult, ALU.add)
            nc.vector.tensor_scalar_max(t1[:], t1[:], 0.0)    # max(S^2 - u^2, 0)
            s_t = smal.tile([P, BT], dt.float32)
            nc.scalar.activation(s_t[:], t1[:], AF.Sqrt, scale=1.0)  # SCALE*sin_t
            t2 = smal.tile([P, BT], dt.float32)
            nc.vector.tensor_scalar_mul(t2[:], s_t[:], -sinm)
            t3 = smal.tile([P, BT], dt.float32)
            nc.vector.tensor_scalar_mul(t3[:], u[:], cosm - 1.0)
            delta = smal.tile([P, BT], dt.float32)
            nc.vector.tensor_add(delta[:], t2[:], t3[:])
            nc.sync.dma_start(dlt_e[:], delta[:])

        # ---------------- main class loop ------------------------------------
        def emit_mms(w_sb, cb, nb):
            pso = po_pool.tile([P, NF], dt.float32, tag="pso", name=f"pso{nb}")
            for k in range(KT):
                nc.tensor.matmul(
                    pso[:],
                    lhsT=w_sb[:, k, cb * P : (cb + 1) * P],
                    rhs=fT[:, k, nb * NF : (nb + 1) * NF],
                    start=(k == 0),
                    stop=(k == KT - 1),
                )
            return pso

        def emit_evac(pso, rnw, cb, nb, osb, cbi, eidx, act_major=True,
                      force_act=False):
            # og1 carries the next chunk's norm blocks, so tilt the
            # evacuation split toward DVE there (ACT holds the squares) —
            # EXCEPT its last c-blocks, whose banks the next og's first
            # matmuls recycle: those go ACT unconditionally so a DVE
            # backlog can't stall the PE across the og boundary
            if force_act or ((eidx % 5 < 3) if act_major else (eidx % 5 >= 2)):
                nc.scalar.activation(
                    osb[:, cbi, nb * NF : (nb + 1) * NF],
                    pso[:], AF.Copy,
                    scale=rnw[:, cb : cb + 1],
                )
            else:
                nc.vector.tensor_scalar_mul(
                    osb[:, cbi, nb * NF : (nb + 1) * NF],
                    pso[:], rnw[:, cb : cb + 1],
                )

        ready = {}   # ci -> (w_sb, rnw)
        raw = {}     # ci -> (w_sb, wn_sb)
        # chunk-0 norms: blocks 0-6 up front (og0's evacuations need them),
        # blocks 7-13 woven into the og0 nb1 pass so they can't queue on ACT
        # ahead of the nb1 evacuations; rsqrt in thirds
        rnw0 = smal.tile([P, CBK], dt.float32, tag="rnw", name="rnw0")
        nw20 = smal.tile([P, CBK], dt.float32, tag="nw2", name="nw20")
        for g in range(7):
            norm_block(wn_sb0, nw20, g)
            if g == 3:
                norm_rsqrt(nw20, rnw0, 0, 4)
        norm_rsqrt(nw20, rnw0, 4, 7)
        ready[0] = (w_sb0, rnw0)
        raw[1] = emit_load(1, rnf20[0:1, 6:7])
        osb_og0 = None  # og0 output tile of the current chunk (prefetch gate)

        for ci in range(CHUNKS):
            w_sb, rnw = ready.pop(ci)
            # norms for the NEXT chunk are spread over this chunk's two
            # output groups: ACT Square blocks (g 0-6) in og0's late slots,
            # DVE affine_mul_reduce blocks (g 7-13) in og1's early slots —
            # never a solid norm burst that starves evacuations and stalls
            # the PE on PSUM-bank recycling. Chunk 1's wn arrives mid-og1,
            # so its norms are emitted in bulk after chunk 0's og1 evacs.
            nxt = None
            if ci + 1 in raw and ci > 0:
                nw_sb, nwn_sb = raw.pop(ci + 1)
                nrnw = smal.tile([P, CBK], dt.float32, tag="rnw", name="rnw")
                nnw2 = smal.tile([P, CBK], dt.float32, tag="nw2", name="nw2")
                nxt = (nw_sb, nwn_sb, nrnw, nnw2)
            for og in range(CBK // OG):
                if og == 0 and ci == 2:
                    # margin input: gated on this chunk's rnw (~mid-kernel)
                    # so the greedy scheduler can't hoist the 1MB transfer
                    # into the startup window
                    gate(wt_sb[0:1, 0:1, 0:1], rnw[0:1, 0:1])
                    nc.gpsimd.dma_start(wt_sb[:], wtg_v)
                if og == 1 and ci == 0 and ci + 1 in raw:
                    nw_sb, nwn_sb = raw.pop(ci + 1)
                    nrnw = smal.tile([P, CBK], dt.float32, tag="rnw", name="rnw")
                    nnw2 = smal.tile([P, CBK], dt.float32, tag="nw2", name="nw2")
                    nxt = (nw_sb, nwn_sb, nrnw, nnw2)
                osb = opool.tile([P, OG, B], dt.bfloat16, tag="osb")
                if ci == 0 and og == 0:
                    # chunk-0 special: all nb=0 matmuls first (they only need
                    # the earlier fT tiles), then the nb=1 pass — the second
                    # batch half's transposes (emitted per-tile above) have
                    # ~12us of nb0 work to complete behind
                    for cbi in range(OG):
                        pso = emit_mms(w_sb, cbi, 0)
                        emit_evac(pso, rnw, cbi, 0, osb, cbi, cbi * NB)
                        if 3 <= cbi < 7:
                            emit_f_transpose(cbi + 1)
                    for cbi in range(OG):
                        pso = emit_mms(w_sb, cbi, 1)
                        emit_evac(pso, rnw, cbi, 1, osb, cbi, cbi * NB + 1)
                        norm_block(wn_sb0, nw20, 7 + cbi)
                    norm_rsqrt(nw20, rnw0, 7, CBK)
                else:
                    for cbi in range(OG):
                        cb = og * OG + cbi
                        pso_pair = [emit_mms(w_sb, cb, nb) for nb in range(NB)]
                        for nb in range(NB):
                            emit_evac(
                                pso_pair[nb], rnw, cb, nb, osb, cbi,
                                (ci * CBK + cb) * NB + nb,
                                act_major=(og == 0 or cbi >= 4),
                            )
                        if nxt is not None and ci > 0 and og == 1:
                            norm_block(nxt[1], nxt[3], 2 * cbi)
                            norm_block(nxt[1], nxt[3], 2 * cbi + 1)
                            if cbi == 3:
                                norm_rsqrt(nxt[3], nxt[2], 0, 7)
                        if og == 0 and ci == 3:
                            emit_margin_dot(cbi)
                        if og == 0 and ci == 4 and cbi == 0:
                            emit_margin_dot(7)
                        if og == 0 and ci == 4 and cbi == 1:
                            emit_margin()
                if og == 1 and nxt is not None:
                    if ci == 0:
                        # bulk after evacs (see above)
                        for g in range(CBK):
                            norm_block(nxt[1], nxt[3], g)
                            if g == 6:
                                norm_rsqrt(nxt[3], nxt[2], 0, 7)
                    norm_rsqrt(nxt[3], nxt[2], 7, CBK)
                    ready[ci + 1] = (nxt[0], nxt[2])
                if og == 0:
                    osb_og0 = osb
                if og == 1 and ci + 2 < CHUNKS:
                    raw[ci + 2] = emit_load(ci + 2, osb_og0[0:1, 0:1, 0:1])
                row0 = ci * CW + og * OG * P
                last_group = ci == CHUNKS - 1 and og == (CBK // OG) - 1
                if last_group:
                    # tail killer: ship the last group per-c-block across
                    # three rings so the kernel ends on a ~260KB transfer
                    engs = [nc.sync, nc.scalar, nc.gpsimd]
                    for cbi in range(OG):
                        engs[cbi % 3].dma_start(
                            out_e[row0 + cbi * P : row0 + (cbi + 1) * P, :]
                            .rearrange("(g p) b -> p g b", p=P),
                            osb[:, cbi : cbi + 1],
                        )
                else:
                    nc.scalar.dma_start(
                        out_e[row0 : row0 + OG * P, :].rearrange(
                            "(g p) b -> p g b", p=P
                        ),
                        osb[:],
                    )

    nc.finalize()
    return nc


def _prep_inputs(features, targets, weights):
    import ml_dtypes

    f32 = np.ascontiguousarray(np.asarray(features, dtype=np.float32))
    tgt = np.asarray(targets).astype(np.int64)
    w = np.asarray(weights, dtype=np.float32)

    wide = (NCORES - 1) * CSTEP + CSH  # 100044
    wpad = np.zeros((wide, IN_F), dtype=np.float32)
    wpad[:OUT_C] = w
    wpad[OUT_C:, 0] = 1.0  # unit-norm filler rows: no inf/nan anywhere

    fb = f32.astype(ml_dtypes.bfloat16)
    in_maps = []
    for i in range(NCORES):
        sh = wpad[i * CSTEP : i * CSTEP + CSH]
        shb = sh.astype(ml_dtypes.bfloat16)
        wT = np.ascontiguousarray(shb.T)
        loc = np.clip(tgt - i * CSTEP, 0, CSH - 1)
        wtgt = np.ascontiguousarray(shb[loc])
        in_maps.append({"wT": wT, "wn": shb, "f": fb, "wtgt": wtgt})
    return in_maps, tgt


def kernel(features, targets, weights):
    global _GRAPH, LAST_EXEC_TIME_NS, LAST_RES
    from concourse.bass_utils import run_bass_kernel_spmd

    if _GRAPH is None:
        _GRAPH = _build_graph()
    nc = _GRAPH

    in_maps, tgt = _prep_inputs(features, targets, weights)

    trace = bool(int(os.environ.get("BASS_KERNEL_TRACE", "0")))
    res = run_bass_kernel_spmd(nc, in_maps, core_ids=list(range(NCORES)), trace=trace)
    LAST_EXEC_TIME_NS = res.exec_time_ns
    LAST_RES = res

    outs = [res.results[i]["out"][:CSTEP] for i in range(NCORES)]  # [12500, B]
    full = np.concatenate(outs, axis=0)                 # [100000, B] bf16
    logits = np.ascontiguousarray(full.T, dtype=np.float32)     # [B, OUT_C] f32

    # apply device-computed margin deltas at the 1024 target positions
    deltas = np.stack(
        [res.results[i]["delta"].T.reshape(B) for i in range(NCORES)]
    )  # [NCORES, B]; delta[p, t] -> b = t*128 + p
    rows = np.arange(B)
    core_of = np.minimum(tgt // CSTEP, NCORES - 1).astype(np.int64)
    logits[rows, tgt] += deltas[core_of, rows]
    return logits


# revision 64
# speedup vs baseline: 1.0492x; 1.0412x over previous
"""ArcFace logits on 8 Trainium2 NeuronCores — class-parallel (partial-FC) sharding.

Math: logits = SCALE * cos(arccos(clip(f_n @ w_n.T)) + MARGIN*onehot(targets))
Since cos(arccos(x)) == x, only the 1024 target entries need the margin
correction cos(t+m) = cos(m)*x - sin(m)*sqrt(1-x^2); everything else is just
the normalized matmul scaled by SCALE.

Device (SPMD, identical graph on all 8 cores, class-sharded; PE does ONLY the
main matmul stream):
  - shards OVERLAP: core i covers classes [i*12500, i*12500+12544) so each
    core runs 98 class-blocks (vs 100 for a disjoint 12800 split) — 2% less
    PE work; host takes [:12500] from each shard
  - startup critical path minimized: f ships as 8 per-tile DMAs on the sync +
    scalar rings, w chunk 0 goes out as 4 cb-ascending pieces on gpsimd, wn
    chunk 0 as 4 pieces on sync/scalar; the f normalize/square work is split
    ACT/DVE per tile and all 32 PE transposes precede the first matmul in the
    PE queue, so the PE starts the main stream as soon as the first w piece
    lands (~4us) with no later transpose stalls
  - w norms: ACT Square+accum_out for even c-blocks, fused DVE
    affine_mul_reduce for odd ones (classes on partitions, exactly the layout
    the evacuation scaling needs); rsqrt in quarters for chunk 0 so the first
    evacuations don't wait on the whole chunk's norms
  - f row-normalize (*SCALE folded in), cast bf16, PE-transpose -> fT
  - main matmul out[c,b] = wT.T @ fT in bf16 (fp32 PSUM); w-norm scaling fused
    into the PSUM->SBUF evacuation (per-partition scale), output cast to bf16
    (rel-err budget is 2e-2; bf16 adds ~2e-3)
  - margin deltas for all 1024 rows from gathered target weight rows, emitted
    mid-loop so they don't sit on the kernel tail
  - output DMAs per 7-cb group alternating sync/scalar rings; the LAST group
    ships per-c-block across three rings so the kernel tail is one ~260KB DMA
Host: shard/transpose/concat + apply the device-computed deltas at the 1024
target positions (pure indexing; all arithmetic happens on device).
"""

import math
import os

import numpy as np

IN_F = 512
OUT_C = 100000
B = 1024
MARGIN = 0.5
SCALE = 20.0

NCORES = 8
CSTEP = OUT_C // NCORES  # 12500 stride between shard starts
CSH = 12544            # classes per core (98 * 128), shards overlap by 44
P = 128
KT = IN_F // P         # 4 contraction subtiles
BT = B // P            # 8 batch tiles
NF = 512               # matmul moving free dim (one PSUM bank of fp32)
NB = B // NF           # 2
CW = 1792              # class chunk width streamed from DRAM (14 c-blocks)
CBK = CW // P          # 14 class blocks per chunk
CHUNKS = CSH // CW     # 7
OG = 7                 # c-blocks batched per output DMA (2 groups per chunk)

_GRAPH = None
LAST_EXEC_TIME_NS = None
LAST_RES = None


def _build_graph():
    from contextlib import ExitStack

    import concourse.bass as bass  # noqa: F401
    import concourse.tile as tile
    from concourse import bacc, mybir
    from concourse.masks import make_identity

    dt = mybir.dt
    AF = mybir.ActivationFunctionType
    ALU = mybir.AluOpType
    cosm = math.cos(MARGIN)
    sinm = math.sin(MARGIN)

    nc = bacc.Bacc()
    wT_e = nc.declare_dram_parameter("wT", [IN_F, CSH], dt.bfloat16, isOutput=False)
    wn_e = nc.declare_dram_parameter("wn", [CSH, IN_F], dt.bfloat16, isOutput=False)
    f_e = nc.declare_dram_parameter("f", [B, IN_F], dt.bfloat16, isOutput=False)
    wtg_e = nc.declare_dram_parameter("wtgt", [B, IN_F], dt.bfloat16, isOutput=False)
    out_e = nc.declare_dram_parameter("out", [CSH, B], dt.bfloat16, isOutput=True)
    dlt_e = nc.declare_dram_parameter("delta", [P, BT], dt.float32, isOutput=True)

    wT_v = wT_e[:].rearrange("(k p) c -> p k c", p=P)   # d = k*128 + p
    wn_v = wn_e[:].rearrange("(n p) d -> p n d", p=P)   # c = n*128 + p
    f_v = f_e[:].rearrange("(t p) d -> p t d", p=P)     # b = t*128 + p
    wtg_v = wtg_e[:].rearrange("(t p) d -> p t d", p=P)

    with ExitStack() as ctx:
        tc = ctx.enter_context(tile.TileContext(nc))
        cpool = ctx.enter_context(tc.tile_pool(name="cpool", bufs=1))
        fpool = ctx.enter_context(tc.tile_pool(name="fpool", bufs=1))
        # bufs=2 is deliberate: chunk ci's DMA then carries a REAL dependency
        # on chunk ci-2's last matmul (pool-buffer recycling), which is the
        # only reliable way to keep the greedy scheduler from hoisting every
        # chunk transfer into the startup window (HBM fair-shares bandwidth
        # across in-flight transfers, starving the critical first tiles)
        wpool = ctx.enter_context(tc.tile_pool(name="wpool", bufs=2))
        wnpool = ctx.enter_context(tc.tile_pool(name="wnpool", bufs=2))
        sqpool = ctx.enter_context(tc.tile_pool(name="sqpool", bufs=3))
        opool = ctx.enter_context(tc.tile_pool(name="opool", bufs=4))
        smal = ctx.enter_context(tc.tile_pool(name="smal", bufs=2))
        # one pool, 7 banks: startup transposes rotate through the same
        # buffers as the matmul accumulators (same tag), the 8th bank is
        # reserved by the framework
        po_pool = ctx.enter_context(tc.tile_pool(name="po", bufs=7, space="PSUM"))

        ident = cpool.tile([P, P], dt.bfloat16)
        make_identity(nc, ident[:])

        # pre-warm the ACT table sets (Sqrt first — it gates rnf20 on the
        # critical path — then Square) during the initial DMA window;
        # otherwise the ~1.5us ACT_TABLE_LOADs land at first real use
        twarm = cpool.tile([P, 1], dt.float32)
        nc.gpsimd.memset(twarm[:], 1.0)
        twout = cpool.tile([P, 1], dt.float32)
        nc.scalar.activation(twout[:], twarm[:], AF.Sqrt, scale=1.0)
        nc.scalar.activation(twout[:], twarm[:], AF.Square)

        # ---------------- DMA priority via per-queue FIFO --------------------
        # Aggregate HBM bandwidth (~358GB/s/core) fair-shares across all
        # queued transfers, so anything queued early delays everything else.
        # Strict ordering: f tiles lead on the sync+scalar rings; the wT
        # stream lives entirely on the gpsimd SWDGE queue in cb-ascending
        # order; wn lives on sync BEHIND f; outputs ride scalar later.
        # A single DMA queue moves only ~25-34GB/s; aggregate bandwidth needs
        # MANY CONCURRENT transfers. First wave: the 8 f tiles fan out across
        # the sync+scalar queue sets (one queue each), wn0's first block gets
        # its own small slot, and the first two w c-blocks ride gpsimd SWDGE
        # queues — everything else is gated behind the f chain.
        f_sb = fpool.tile([P, BT, IN_F], dt.bfloat16)
        w_sb0 = wpool.tile([P, KT, CW], dt.bfloat16, tag="wchunk", name="w_sb0")
        wn_sb0 = wnpool.tile([P, CBK, IN_F], dt.bfloat16, tag="wnchunk", name="wn_sb0")
        for t in (0, 2, 4, 6):
            nc.sync.dma_start(f_sb[:, t], f_v[:, t])
        for t in (1, 3, 5, 7):
            nc.scalar.dma_start(f_sb[:, t], f_v[:, t])
        nc.gpsimd.dma_start(w_sb0[:, :, 0:P], wT_v[:, :, 0:P])
        nc.gpsimd.dma_start(wn_sb0[:, 0:1], wn_v[:, 0:1])
        nc.gpsimd.dma_start(w_sb0[:, :, P : 2 * P], wT_v[:, :, P : 2 * P])
        nc.gpsimd.dma_start(wn_sb0[:, 1:4], wn_v[:, 1:4])

        # ---------------- f normalize + transpose, pipelined per tile -------
        # First half all on DVE (affine_mul_reduce needs no ACT table; the
        # two ACT table loads take ~2.6us and would gate the chain).
        nf2 = smal.tile([P, BT], dt.float32)
        rnf20 = smal.tile([P, BT], dt.float32)
        f_n = fpool.tile([P, BT, IN_F], dt.bfloat16)
        fT = fpool.tile([P, KT, B], dt.bfloat16)

        def emit_f_tile(t):
            # fully per-tile pipeline: square -> recip -> sqrt -> normalize
            # -> 4 transposes -> fT copy. No cross-tile barrier — the old
            # half-granular reciprocal waited on ALL four squares, a
            # dependency the greedy scheduler could not break.
            sq = sqpool.tile([P, IN_F], dt.bfloat16, tag="sqscratch")
            if t % 2 == 0:
                nc.vector.affine_mul_reduce(
                    sq[:], nf2[:, t : t + 1], f_sb[:, t], f_sb[:, t], 1.0, 0.0
                )
            else:
                nc.scalar.activation(
                    sq[:], f_sb[:, t], AF.Square, accum_out=nf2[:, t : t + 1]
                )
            rec = smal.tile([P, 1], dt.float32, tag="recf")
            nc.vector.reciprocal(rec[:], nf2[:, t : t + 1])
            # sqrt(SCALE^2 / nf2) = SCALE * rsqrt(nf2)
            nc.scalar.activation(
                rnf20[:, t : t + 1], rec[:], AF.Sqrt, scale=SCALE * SCALE
            )
            nc.vector.tensor_scalar_mul(f_n[:, t], f_sb[:, t], rnf20[:, t : t + 1])

        def emit_f_transpose(t):
            ps = po_pool.tile([P, KT, P], dt.bfloat16, tag="pso")
            for k in range(KT):
                nc.tensor.transpose(
                    ps[:, k], f_n[:, t, k * P : (k + 1) * P], ident[:]
                )
            if t % 2 == 0:
                nc.vector.tensor_copy(fT[:, :, t * P : (t + 1) * P], ps[:])
            else:
                nc.scalar.copy(fT[:, :, t * P : (t + 1) * P], ps[:])

        for t in range(BT):
            emit_f_tile(t)
        for t in range(4):
            # t4-7 transposes are deferred into the og0-special loop: their
            # f chains drain behind chunk-0 norm work, and an early PE-queue
            # slot would stall the matmul stream on them
            emit_f_transpose(t)

        def gate(dst, src):
            # tiny gpsimd write into a DMA's destination region: the DMA
            # inherits a write-after-write dependency, i.e. a REAL "not
            # before src is ready" gate the scheduler can't hoist past;
            # gpsimd so the gate never head-of-line-blocks ACT/DVE.
            # ONLY safe on a fresh (never-written) tile — a mid-tile write
            # after earlier DMAs makes the framework duplicate the DMA.
            nc.gpsimd.tensor_copy(dst, src)

        gscr = cpool.tile([1, 1], dt.float32)

        def gate_r(region, src):
            # WAR variant for partially-written tiles: READ the destination
            # region (+ the gate source) so the DMA waits on the reader.
            # DVE (gpsimd can't codegen a generic TensorTensor); the wait is
            # on rnf20 which the DVE queue has just produced, so no HOL risk
            nc.vector.tensor_tensor(gscr[:], src, region, op=ALU.bypass)

        # chunk-0 bulk rides ungated right behind cb0-1 on the gpsimd queue;
        # wn0's tail is gated on the first-half f squares so the earliest
        # window carries only ~2MB of critical transfers
        nc.gpsimd.dma_start(w_sb0[:, :, 2 * P : 7 * P], wT_v[:, :, 2 * P : 7 * P])
        nc.gpsimd.dma_start(w_sb0[:, :, 7 * P : CW], wT_v[:, :, 7 * P : CW])
        gate_r(wn_sb0[0:1, 4:5, 0:1], nf2[0:1, 3:4])
        nc.sync.dma_start(wn_sb0[:, 4:CBK], wn_v[:, 4:CBK])

        # ---------------- w chunk streams (steady state) --------------------
        # chunk ci's transfers are gated on chunk ci-2's first output-group
        # tile (written mid-ci-2), giving each chunk ~1.5 windows of
        # transfer time without polluting the startup burst
        def emit_load(ci, gsrc):
            w_sb = wpool.tile([P, KT, CW], dt.bfloat16, tag="wchunk", name="w_sb")
            wn_sb = wnpool.tile([P, CBK, IN_F], dt.bfloat16, tag="wnchunk", name="wn_sb")
            half = (CBK // 2) * P
            gate(w_sb[0:1, 0:1, 0:1], gsrc)
            nc.gpsimd.dma_start(
                w_sb[:, :, :half], wT_v[:, :, ci * CW : ci * CW + half]
            )
            gate(w_sb[0:1, 0:1, half : half + 1], gsrc)
            nc.gpsimd.dma_start(
                w_sb[:, :, half:], wT_v[:, :, ci * CW + half : (ci + 1) * CW]
            )
            gate(wn_sb[0:1, 0:1, 0:1], gsrc)
            nc.sync.dma_start(wn_sb[:], wn_v[:, ci * CBK : (ci + 1) * CBK])
            return w_sb, wn_sb

        def emit_norm(wn_sb, first=False):
            """Per-class 1/||w|| for one chunk -> [128, CBK], classes on
            partitions. ACT Square+accum for even blocks, fused DVE
            affine_mul_reduce for odd; rsqrt in quarters (chunk 0) or halves
            so early evacuations don't wait on the whole chunk's norms."""
            rnw = smal.tile([P, CBK], dt.float32, tag="rnw", name="rnw")
            nw2 = smal.tile([P, CBK], dt.float32, tag="nw2", name="nw2")
            bounds = [4, 8, 11, CBK] if first else [CBK // 2, CBK]
            done = 0
            for g in range(CBK):
                norm_block(wn_sb, nw2, g)
                if g + 1 in bounds:
                    norm_rsqrt(nw2, rnw, done, g + 1)
                    done = g + 1
            return rnw

        def norm_block(wn_sb, nw2, g, act=None):
            sq = sqpool.tile([P, IN_F], dt.bfloat16, tag="sqscratch")
            if (g % 2 == 0) if act is None else act:
                nc.scalar.activation(
                    sq[:], wn_sb[:, g], AF.Square, accum_out=nw2[:, g : g + 1]
                )
            else:
                nc.vector.affine_mul_reduce(
                    sq[:], nw2[:, g : g + 1], wn_sb[:, g], wn_sb[:, g], 1.0, 0.0
                )

        def norm_rsqrt(nw2, rnw, a, b):
            recw = smal.tile([P, b - a], dt.float32, tag=f"recw{a}", name="recw")
            nc.vector.reciprocal(recw[:], nw2[:, a:b])
            nc.scalar.activation(rnw[:, a:b], recw[:], AF.Sqrt, scale=1.0)

        # margin input DMA deferred to mid-loop
        wt_sb = fpool.tile([P, BT, IN_F], dt.bfloat16, name="wt_sb")
        nt2 = smal.tile([P, BT], dt.float32, name="nt2")
        drot = smal.tile([P, BT], dt.float32, name="drot")

        def emit_margin_dot(t):
            sq = sqpool.tile([P, IN_F], dt.bfloat16, tag="sqscratch")
            nc.scalar.activation(
                sq[:], wt_sb[:, t], AF.Square, accum_out=nt2[:, t : t + 1]
            )
            prod = sqpool.tile([P, IN_F], dt.bfloat16, tag="sqscratch")
            nc.vector.affine_mul_reduce(
                prod[:], drot[:, t : t + 1], f_sb[:, t], wt_sb[:, t], 1.0, 0.0
            )

        def emit_margin():
            rec_t = smal.tile([P, BT], dt.float32)
            nc.vector.reciprocal(rec_t[:], nt2[:])
            rnt = smal.tile([P, BT], dt.float32)
            nc.scalar.activation(rnt[:], rec_t[:], AF.Sqrt, scale=1.0)
            u = smal.tile([P, BT], dt.float32)
            nc.vector.tensor_mul(u[:], drot[:], rnf20[:])
            nc.vector.tensor_mul(u[:], u[:], rnt[:])          # u = SCALE * cos_t
            t1 = smal.tile([P, BT], dt.float32)
            nc.vector.tensor_mul(t1[:], u[:], u[:])
            nc.vector.tensor_scalar(t1[:], t1[:], -1.0, SCALE * SCALE, ALU.mult, ALU.add)
            nc.vector.tensor_scalar_max(t1[:], t1[:], 0.0)    # max(S^2 - u^2, 0)
            s_t = smal.tile([P, BT], dt.float32)
            nc.scalar.activation(s_t[:], t1[:], AF.Sqrt, scale=1.0)  # SCALE*sin_t
            t2 = smal.tile([P, BT], dt.float32)
            nc.vector.tensor_scalar_mul(t2[:], s_t[:], -sinm)
            t3 = smal.tile([P, BT], dt.float32)
            nc.vector.tensor_scalar_mul(t3[:], u[:], cosm - 1.0)
            delta = smal.tile([P, BT], dt.float32)
            nc.vector.tensor_add(delta[:], t2[:], t3[:])
            nc.sync.dma_start(dlt_e[:], delta[:])

        # ---------------- main class loop ------------------------------------
        def emit_mms(w_sb, cb, nb):
            pso = po_pool.tile([P, NF], dt.float32, tag="pso", name=f"pso{nb}")
            for k in range(KT):
                nc.tensor.matmul(
                    pso[:],
                    lhsT=w_sb[:, k, cb * P : (cb + 1) * P],
                    rhs=fT[:, k, nb * NF : (nb + 1) * NF],
                    start=(k == 0),
                    stop=(k == KT - 1),
                )
            return pso

        def emit_evac(pso, rnw, cb, nb, osb, cbi, eidx, act_major=True,
                      force_act=False):
            # og1 carries the next chunk's norm blocks, so tilt the
            # evacuation split toward DVE there (ACT holds the squares) —
            # EXCEPT its last c-blocks, whose banks the next og's first
            # matmuls recycle: those go ACT unconditionally so a DVE
            # backlog can't stall the PE across the og boundary
            if force_act or ((eidx % 5 < 3) if act_major else (eidx % 5 >= 2)):
                nc.scalar.activation(
                    osb[:, cbi, nb * NF : (nb + 1) * NF],
                    pso[:], AF.Copy,
                    scale=rnw[:, cb : cb + 1],
                )
            else:
                nc.vector.tensor_scalar_mul(
                    osb[:, cbi, nb * NF : (nb + 1) * NF],
                    pso[:], rnw[:, cb : cb + 1],
                )

        ready = {}   # ci -> (w_sb, rnw)
        raw = {}     # ci -> (w_sb, wn_sb)
        # chunk-0 norms: blocks 0-6 up front (og0's evacuations need them),
        # blocks 7-13 woven into the og0 nb1 pass so they can't queue on ACT
        # ahead of the nb1 evacuations; rsqrt in thirds
        rnw0 = smal.tile([P, CBK], dt.float32, tag="rnw", name="rnw0")
        nw20 = smal.tile([P, CBK], dt.float32, tag="nw2", name="nw20")
        for g in range(7):
            norm_block(wn_sb0, nw20, g)
            if g == 3:
                norm_rsqrt(nw20, rnw0, 0, 4)
        norm_rsqrt(nw20, rnw0, 4, 7)
        ready[0] = (w_sb0, rnw0)
        raw[1] = emit_load(1, rnf20[0:1, 6:7])
        osb_og0 = None  # og0 output tile of the current chunk (prefetch gate)

        for ci in range(CHUNKS):
            w_sb, rnw = ready.pop(ci)
            # norms for the NEXT chunk are spread over this chunk's two
            # output groups: ACT Square blocks (g 0-6) in og0's late slots,
            # DVE affine_mul_reduce blocks (g 7-13) in og1's early slots —
            # never a solid norm burst that starves evacuations and stalls
            # the PE on PSUM-bank recycling. Chunk 1's wn arrives mid-og1,
            # so its norms are emitted in bulk after chunk 0's og1 evacs.
            nxt = None
            if ci + 1 in raw and ci > 0:
                nw_sb, nwn_sb = raw.pop(ci + 1)
                nrnw = smal.tile([P, CBK], dt.float32, tag="rnw", name="rnw")
                nnw2 = smal.tile([P, CBK], dt.float32, tag="nw2", name="nw2")
                nxt = (nw_sb, nwn_sb, nrnw, nnw2)
            for og in range(CBK // OG):
                if og == 0 and ci == 2:
                    # margin input: gated on this chunk's rnw (~mid-kernel)
                    # so the greedy scheduler can't hoist the 1MB transfer
                    # into the startup window
                    gate(wt_sb[0:1, 0:1, 0:1], rnw[0:1, 0:1])
                    nc.gpsimd.dma_start(wt_sb[:], wtg_v)
                if og == 1 and ci == 0 and ci + 1 in raw:
                    nw_sb, nwn_sb = raw.pop(ci + 1)
                    nrnw = smal.tile([P, CBK], dt.float32, tag="rnw", name="rnw")
                    nnw2 = smal.tile([P, CBK], dt.float32, tag="nw2", name="nw2")
                    nxt = (nw_sb, nwn_sb, nrnw, nnw2)
                osb = opool.tile([P, OG, B], dt.bfloat16, tag="osb")
                if ci == 0 and og == 0:
                    # chunk-0 special: all nb=0 matmuls first (they only need
                    # the earlier fT tiles), then the nb=1 pass — the second
                    # batch half's transposes (emitted per-tile above) have
                    # ~12us of nb0 work to complete behind
                    for cbi in range(OG):
                        pso = emit_mms(w_sb, cbi, 0)
                        emit_evac(pso, rnw, cbi, 0, osb, cbi, cbi * NB)
                        if 3 <= cbi < 7:
                            emit_f_transpose(cbi + 1)
                    for cbi in range(OG):
                        pso = emit_mms(w_sb, cbi, 1)
                        emit_evac(pso, rnw, cbi, 1, osb, cbi, cbi * NB + 1)
                        norm_block(wn_sb0, nw20, 7 + cbi)
                    norm_rsqrt(nw20, rnw0, 7, CBK)
                else:
                    for cbi in range(OG):
                        cb = og * OG + cbi
                        pso_pair = [emit_mms(w_sb, cb, nb) for nb in range(NB)]
                        for nb in range(NB):
                            emit_evac(
                                pso_pair[nb], rnw, cb, nb, osb, cbi,
                                (ci * CBK + cb) * NB + nb,
                                act_major=(og == 0 or cbi >= 4),
                            )
                        if nxt is not None and ci > 0 and og == 1:
                            norm_block(nxt[1], nxt[3], 2 * cbi)
                            norm_block(nxt[1], nxt[3], 2 * cbi + 1)
                            if cbi == 3:
                                norm_rsqrt(nxt[3], nxt[2], 0, 7)
                        if og == 0 and ci == 3:
                            emit_margin_dot(cbi)
                        if og == 0 and ci == 4 and cbi == 0:
                            emit_margin_dot(7)
                        if og == 0 and ci == 4 and cbi == 1:
                            emit_margin()
                if og == 1 and nxt is not None:
                    if ci == 0:
                        # bulk after evacs (see above)
                        for g in range(CBK):
                            norm_block(nxt[1], nxt[3], g)
                            if g == 6:
                                norm_rsqrt(nxt[3], nxt[2], 0, 7)
                    norm_rsqrt(nxt[3], nxt[2], 7, CBK)
                    ready[ci + 1] = (nxt[0], nxt[2])
                if og == 0:
                    osb_og0 = osb
                if og == 1 and ci + 2 < CHUNKS:
                    raw[ci + 2] = emit_load(ci + 2, osb_og0[0:1, 0:1, 0:1])
                row0 = ci * CW + og * OG * P
                last_group = ci == CHUNKS - 1 and og == (CBK // OG) - 1
                if last_group:
                    # tail killer: ship the last group per-c-block across
                    # three rings so the kernel ends on a ~260KB transfer
                    engs = [nc.sync, nc.scalar, nc.gpsimd]
                    for cbi in range(OG):
                        engs[cbi % 3].dma_start(
                            out_e[row0 + cbi * P : row0 + (cbi + 1) * P, :]
                            .rearrange("(g p) b -> p g b", p=P),
                            osb[:, cbi : cbi + 1],
                        )
                else:
                    # alternate output rings: one queue moves only ~25-34GB/s,
                    # and a backed-up output stream delays osb buffer recycling
                    # -> evacuations -> PSUM banks -> the PE itself
                    eng = nc.scalar if (ci * 2 + og) % 2 == 0 else nc.sync
                    eng.dma_start(
                        out_e[row0 : row0 + OG * P, :].rearrange(
                            "(g p) b -> p g b", p=P
                        ),
                        osb[:],
                    )

    nc.finalize()
    return nc


def _prep_inputs(features, targets, weights):
    import ml_dtypes

    f32 = np.ascontiguousarray(np.asarray(features, dtype=np.float32))
    tgt = np.asarray(targets).astype(np.int64)
    w = np.asarray(weights, dtype=np.float32)

    wide = (NCORES - 1) * CSTEP + CSH  # 100044
    wpad = np.zeros((wide, IN_F), dtype=np.float32)
    wpad[:OUT_C] = w
    wpad[OUT_C:, 0] = 1.0  # unit-norm filler rows: no inf/nan anywhere

    fb = f32.astype(ml_dtypes.bfloat16)
    in_maps = []
    for i in range(NCORES):
        sh = wpad[i * CSTEP : i * CSTEP + CSH]
        shb = sh.astype(ml_dtypes.bfloat16)
        wT = np.ascontiguousarray(shb.T)
        loc = np.clip(tgt - i * CSTEP, 0, CSH - 1)
        wtgt = np.ascontiguousarray(shb[loc])
        in_maps.append({"wT": wT, "wn": shb, "f": fb, "wtgt": wtgt})
    return in_maps, tgt


def kernel(features, targets, weights):
    global _GRAPH, LAST_EXEC_TIME_NS, LAST_RES
    from concourse.bass_utils import run_bass_kernel_spmd

    if _GRAPH is None:
        _GRAPH = _build_graph()
    nc = _GRAPH

    in_maps, tgt = _prep_inputs(features, targets, weights)

    trace = bool(int(os.environ.get("BASS_KERNEL_TRACE", "0")))
    res = run_bass_kernel_spmd(nc, in_maps, core_ids=list(range(NCORES)), trace=trace)
    LAST_EXEC_TIME_NS = res.exec_time_ns
    LAST_RES = res

    outs = [res.results[i]["out"][:CSTEP] for i in range(NCORES)]  # [12500, B]
    full = np.concatenate(outs, axis=0)                 # [100000, B] bf16
    logits = np.ascontiguousarray(full.T, dtype=np.float32)     # [B, OUT_C] f32

    # apply device-computed margin deltas at the 1024 target positions
    deltas = np.stack(
        [res.results[i]["delta"].T.reshape(B) for i in range(NCORES)]
    )  # [NCORES, B]; delta[p, t] -> b = t*128 + p
    rows = np.arange(B)
    core_of = np.minimum(tgt // CSTEP, NCORES - 1).astype(np.int64)
    logits[rows, tgt] += deltas[core_of, rows]
    return logits


# revision 65
# speedup vs baseline: 1.0530x; 1.0036x over previous
"""ArcFace logits on 8 Trainium2 NeuronCores — class-parallel (partial-FC) sharding.

Math: logits = SCALE * cos(arccos(clip(f_n @ w_n.T)) + MARGIN*onehot(targets))
Since cos(arccos(x)) == x, only the 1024 target entries need the margin
correction cos(t+m) = cos(m)*x - sin(m)*sqrt(1-x^2); everything else is just
the normalized matmul scaled by SCALE.

Device (SPMD, identical graph on all 8 cores, class-sharded; PE does ONLY the
main matmul stream):
  - shards OVERLAP: core i covers classes [i*12500, i*12500+12544) so each
    core runs 98 class-blocks (vs 100 for a disjoint 12800 split) — 2% less
    PE work; host takes [:12500] from each shard
  - startup critical path minimized: f ships as 8 per-tile DMAs on the sync +
    scalar rings, w chunk 0 goes out as 4 cb-ascending pieces on gpsimd, wn
    chunk 0 as 4 pieces on sync/scalar; the f normalize/square work is split
    ACT/DVE per tile and all 32 PE transposes precede the first matmul in the
    PE queue, so the PE starts the main stream as soon as the first w piece
    lands (~4us) with no later transpose stalls
  - w norms: ACT Square+accum_out for even c-blocks, fused DVE
    affine_mul_reduce for odd ones (classes on partitions, exactly the layout
    the evacuation scaling needs); rsqrt in quarters for chunk 0 so the first
    evacuations don't wait on the whole chunk's norms
  - f row-normalize (*SCALE folded in), cast bf16, PE-transpose -> fT
  - main matmul out[c,b] = wT.T @ fT in bf16 (fp32 PSUM); w-norm scaling fused
    into the PSUM->SBUF evacuation (per-partition scale), output cast to bf16
    (rel-err budget is 2e-2; bf16 adds ~2e-3)
  - margin deltas for all 1024 rows from gathered target weight rows, emitted
    mid-loop so they don't sit on the kernel tail
  - output DMAs per 7-cb group alternating sync/scalar rings; the LAST group
    ships per-c-block across three rings so the kernel tail is one ~260KB DMA
Host: shard/transpose/concat + apply the device-computed deltas at the 1024
target positions (pure indexing; all arithmetic happens on device).
"""

import math
import os

import numpy as np

IN_F = 512
OUT_C = 100000
B = 1024
MARGIN = 0.5
SCALE = 20.0

NCORES = 8
CSTEP = OUT_C // NCORES  # 12500 stride between shard starts
CSH = 12544            # classes per core (98 * 128), shards overlap by 44
P = 128
KT = IN_F // P         # 4 contraction subtiles
BT = B // P            # 8 batch tiles
NF = 512               # matmul moving free dim (one PSUM bank of fp32)
NB = B // NF           # 2
CW = 1792              # class chunk width streamed from DRAM (14 c-blocks)
CBK = CW // P          # 14 class blocks per chunk
CHUNKS = CSH // CW     # 7
OG = 7                 # c-blocks batched per output DMA (2 groups per chunk)

_GRAPH = None
LAST_EXEC_TIME_NS = None
LAST_RES = None


def _build_graph():
    from contextlib import ExitStack

    import concourse.bass as bass  # noqa: F401
    import concourse.tile as tile
    from concourse import bacc, mybir
    from concourse.masks import make_identity

    dt = mybir.dt
    AF = mybir.ActivationFunctionType
    ALU = mybir.AluOpType
    cosm = math.cos(MARGIN)
    sinm = math.sin(MARGIN)

    nc = bacc.Bacc()
    wT_e = nc.declare_dram_parameter("wT", [IN_F, CSH], dt.bfloat16, isOutput=False)
    wn_e = nc.declare_dram_parameter("wn", [CSH, IN_F], dt.bfloat16, isOutput=False)
    f_e = nc.declare_dram_parameter("f", [B, IN_F], dt.bfloat16, isOutput=False)
    wtg_e = nc.declare_dram_parameter("wtgt", [B, IN_F], dt.bfloat16, isOutput=False)
    out_e = nc.declare_dram_parameter("out", [CSH, B], dt.bfloat16, isOutput=True)
    dlt_e = nc.declare_dram_parameter("delta", [P, BT], dt.float32, isOutput=True)

    wT_v = wT_e[:].rearrange("(k p) c -> p k c", p=P)   # d = k*128 + p
    wn_v = wn_e[:].rearrange("(n p) d -> p n d", p=P)   # c = n*128 + p
    f_v = f_e[:].rearrange("(t p) d -> p t d", p=P)     # b = t*128 + p
    wtg_v = wtg_e[:].rearrange("(t p) d -> p t d", p=P)

    with ExitStack() as ctx:
        tc = ctx.enter_context(tile.TileContext(nc))
        cpool = ctx.enter_context(tc.tile_pool(name="cpool", bufs=1))
        fpool = ctx.enter_context(tc.tile_pool(name="fpool", bufs=1))
        # bufs=2 is deliberate: chunk ci's DMA then carries a REAL dependency
        # on chunk ci-2's last matmul (pool-buffer recycling), which is the
        # only reliable way to keep the greedy scheduler from hoisting every
        # chunk transfer into the startup window (HBM fair-shares bandwidth
        # across in-flight transfers, starving the critical first tiles)
        wpool = ctx.enter_context(tc.tile_pool(name="wpool", bufs=2))
        wnpool = ctx.enter_context(tc.tile_pool(name="wnpool", bufs=2))
        sqpool = ctx.enter_context(tc.tile_pool(name="sqpool", bufs=3))
        opool = ctx.enter_context(tc.tile_pool(name="opool", bufs=4))
        smal = ctx.enter_context(tc.tile_pool(name="smal", bufs=2))
        # one pool, 7 banks: startup transposes rotate through the same
        # buffers as the matmul accumulators (same tag), the 8th bank is
        # reserved by the framework
        po_pool = ctx.enter_context(tc.tile_pool(name="po", bufs=7, space="PSUM"))

        ident = cpool.tile([P, P], dt.bfloat16)
        make_identity(nc, ident[:])

        # pre-warm the ACT table sets (Sqrt first — it gates rnf20 on the
        # critical path — then Square) during the initial DMA window;
        # otherwise the ~1.5us ACT_TABLE_LOADs land at first real use
        twarm = cpool.tile([P, 1], dt.float32)
        nc.gpsimd.memset(twarm[:], 1.0)
        twout = cpool.tile([P, 1], dt.float32)
        nc.scalar.activation(twout[:], twarm[:], AF.Sqrt, scale=1.0)
        nc.scalar.activation(twout[:], twarm[:], AF.Square)

        # ---------------- DMA priority via per-queue FIFO --------------------
        # Aggregate HBM bandwidth (~358GB/s/core) fair-shares across all
        # queued transfers, so anything queued early delays everything else.
        # Strict ordering: f tiles lead on the sync+scalar rings; the wT
        # stream lives entirely on the gpsimd SWDGE queue in cb-ascending
        # order; wn lives on sync BEHIND f; outputs ride scalar later.
        # A single DMA queue moves only ~25-34GB/s; aggregate bandwidth needs
        # MANY CONCURRENT transfers. First wave: the 8 f tiles fan out across
        # the sync+scalar queue sets (one queue each), wn0's first block gets
        # its own small slot, and the first two w c-blocks ride gpsimd SWDGE
        # queues — everything else is gated behind the f chain.
        f_sb = fpool.tile([P, BT, IN_F], dt.bfloat16)
        w_sb0 = wpool.tile([P, KT, CW], dt.bfloat16, tag="wchunk", name="w_sb0")
        wn_sb0 = wnpool.tile([P, CBK, IN_F], dt.bfloat16, tag="wnchunk", name="wn_sb0")
        for t in (0, 2, 4, 6):
            nc.sync.dma_start(f_sb[:, t], f_v[:, t])
        for t in (1, 3, 5, 7):
            nc.scalar.dma_start(f_sb[:, t], f_v[:, t])
        nc.gpsimd.dma_start(w_sb0[:, :, 0:P], wT_v[:, :, 0:P])
        nc.gpsimd.dma_start(wn_sb0[:, 0:1], wn_v[:, 0:1])
        nc.gpsimd.dma_start(w_sb0[:, :, P : 2 * P], wT_v[:, :, P : 2 * P])
        nc.gpsimd.dma_start(wn_sb0[:, 1:4], wn_v[:, 1:4])

        # ---------------- f normalize + transpose, pipelined per tile -------
        # First half all on DVE (affine_mul_reduce needs no ACT table; the
        # two ACT table loads take ~2.6us and would gate the chain).
        nf2 = smal.tile([P, BT], dt.float32)
        rnf20 = smal.tile([P, BT], dt.float32)
        f_n = fpool.tile([P, BT, IN_F], dt.bfloat16)
        fT = fpool.tile([P, KT, B], dt.bfloat16)

        def emit_f_tile(t):
            # fully per-tile pipeline: square -> recip -> sqrt -> normalize
            # -> 4 transposes -> fT copy. No cross-tile barrier — the old
            # half-granular reciprocal waited on ALL four squares, a
            # dependency the greedy scheduler could not break.
            sq = sqpool.tile([P, IN_F], dt.bfloat16, tag="sqscratch")
            if t % 2 == 0:
                nc.vector.affine_mul_reduce(
                    sq[:], nf2[:, t : t + 1], f_sb[:, t], f_sb[:, t], 1.0, 0.0
                )
            else:
                nc.scalar.activation(
                    sq[:], f_sb[:, t], AF.Square, accum_out=nf2[:, t : t + 1]
                )
            rec = smal.tile([P, 1], dt.float32, tag="recf")
            nc.vector.reciprocal(rec[:], nf2[:, t : t + 1])
            # sqrt(SCALE^2 / nf2) = SCALE * rsqrt(nf2)
            nc.scalar.activation(
                rnf20[:, t : t + 1], rec[:], AF.Sqrt, scale=SCALE * SCALE
            )
            nc.vector.tensor_scalar_mul(f_n[:, t], f_sb[:, t], rnf20[:, t : t + 1])

        def emit_f_transpose(t):
            ps = po_pool.tile([P, KT, P], dt.bfloat16, tag="pso")
            for k in range(KT):
                nc.tensor.transpose(
                    ps[:, k], f_n[:, t, k * P : (k + 1) * P], ident[:]
                )
            if t % 2 == 0:
                nc.vector.tensor_copy(fT[:, :, t * P : (t + 1) * P], ps[:])
            else:
                nc.scalar.copy(fT[:, :, t * P : (t + 1) * P], ps[:])

        for t in range(BT):
            emit_f_tile(t)
        for t in range(4):
            # t4-7 transposes are deferred into the og0-special loop: their
            # f chains drain behind chunk-0 norm work, and an early PE-queue
            # slot would stall the matmul stream on them
            emit_f_transpose(t)

        def gate(dst, src):
            # tiny gpsimd write into a DMA's destination region: the DMA
            # inherits a write-after-write dependency, i.e. a REAL "not
            # before src is ready" gate the scheduler can't hoist past;
            # gpsimd so the gate never head-of-line-blocks ACT/DVE.
            # ONLY safe on a fresh (never-written) tile — a mid-tile write
            # after earlier DMAs makes the framework duplicate the DMA.
            nc.gpsimd.tensor_copy(dst, src)

        gscr = cpool.tile([1, 1], dt.float32)

        def gate_r(region, src):
            # WAR variant for partially-written tiles: READ the destination
            # region (+ the gate source) so the DMA waits on the reader.
            # DVE (gpsimd can't codegen a generic TensorTensor); the wait is
            # on rnf20 which the DVE queue has just produced, so no HOL risk
            nc.vector.tensor_tensor(gscr[:], src, region, op=ALU.bypass)

        # chunk-0 bulk rides ungated right behind cb0-1 on the gpsimd queue;
        # wn0's tail is gated on the first-half f squares so the earliest
        # window carries only ~2MB of critical transfers
        nc.gpsimd.dma_start(w_sb0[:, :, 2 * P : 7 * P], wT_v[:, :, 2 * P : 7 * P])
        nc.gpsimd.dma_start(w_sb0[:, :, 7 * P : CW], wT_v[:, :, 7 * P : CW])
        gate_r(wn_sb0[0:1, 4:5, 0:1], nf2[0:1, 3:4])
        nc.sync.dma_start(wn_sb0[:, 4:CBK], wn_v[:, 4:CBK])

        # ---------------- w chunk streams (steady state) --------------------
        # chunk ci's transfers are gated on chunk ci-2's first output-group
        # tile (written mid-ci-2), giving each chunk ~1.5 windows of
        # transfer time without polluting the startup burst
        def emit_load(ci, gsrc):
            w_sb = wpool.tile([P, KT, CW], dt.bfloat16, tag="wchunk", name="w_sb")
            wn_sb = wnpool.tile([P, CBK, IN_F], dt.bfloat16, tag="wnchunk", name="wn_sb")
            half = (CBK // 2) * P
            gate(w_sb[0:1, 0:1, 0:1], gsrc)
            nc.gpsimd.dma_start(
                w_sb[:, :, :half], wT_v[:, :, ci * CW : ci * CW + half]
            )
            gate(w_sb[0:1, 0:1, half : half + 1], gsrc)
            nc.gpsimd.dma_start(
                w_sb[:, :, half:], wT_v[:, :, ci * CW + half : (ci + 1) * CW]
            )
            gate(wn_sb[0:1, 0:1, 0:1], gsrc)
            nc.sync.dma_start(wn_sb[:], wn_v[:, ci * CBK : (ci + 1) * CBK])
            return w_sb, wn_sb

        def emit_norm(wn_sb, first=False):
            """Per-class 1/||w|| for one chunk -> [128, CBK], classes on
            partitions. ACT Square+accum for even blocks, fused DVE
            affine_mul_reduce for odd; rsqrt in quarters (chunk 0) or halves
            so early evacuations don't wait on the whole chunk's norms."""
            rnw = smal.tile([P, CBK], dt.float32, tag="rnw", name="rnw")
            nw2 = smal.tile([P, CBK], dt.float32, tag="nw2", name="nw2")
            bounds = [4, 8, 11, CBK] if first else [CBK // 2, CBK]
            done = 0
            for g in range(CBK):
                norm_block(wn_sb, nw2, g)
                if g + 1 in bounds:
                    norm_rsqrt(nw2, rnw, done, g + 1)
                    done = g + 1
            return rnw

        def norm_block(wn_sb, nw2, g, act=None):
            sq = sqpool.tile([P, IN_F], dt.bfloat16, tag="sqscratch")
            if (g % 2 == 0) if act is None else act:
                nc.scalar.activation(
                    sq[:], wn_sb[:, g], AF.Square, accum_out=nw2[:, g : g + 1]
                )
            else:
                nc.vector.affine_mul_reduce(
                    sq[:], nw2[:, g : g + 1], wn_sb[:, g], wn_sb[:, g], 1.0, 0.0
                )

        def norm_rsqrt(nw2, rnw, a, b):
            recw = smal.tile([P, b - a], dt.float32, tag=f"recw{a}", name="recw")
            nc.vector.reciprocal(recw[:], nw2[:, a:b])
            nc.scalar.activation(rnw[:, a:b], recw[:], AF.Sqrt, scale=1.0)

        # margin input DMA deferred to mid-loop
        wt_sb = fpool.tile([P, BT, IN_F], dt.bfloat16, name="wt_sb")
        nt2 = smal.tile([P, BT], dt.float32, name="nt2")
        drot = smal.tile([P, BT], dt.float32, name="drot")

        def emit_margin_dot(t):
            sq = sqpool.tile([P, IN_F], dt.bfloat16, tag="sqscratch")
            nc.scalar.activation(
                sq[:], wt_sb[:, t], AF.Square, accum_out=nt2[:, t : t + 1]
            )
            prod = sqpool.tile([P, IN_F], dt.bfloat16, tag="sqscratch")
            nc.vector.affine_mul_reduce(
                prod[:], drot[:, t : t + 1], f_sb[:, t], wt_sb[:, t], 1.0, 0.0
            )

        def emit_margin():
            rec_t = smal.tile([P, BT], dt.float32)
            nc.vector.reciprocal(rec_t[:], nt2[:])
            rnt = smal.tile([P, BT], dt.float32)
            nc.scalar.activation(rnt[:], rec_t[:], AF.Sqrt, scale=1.0)
            u = smal.tile([P, BT], dt.float32)
            nc.vector.tensor_mul(u[:], drot[:], rnf20[:])
            nc.vector.tensor_mul(u[:], u[:], rnt[:])          # u = SCALE * cos_t
            t1 = smal.tile([P, BT], dt.float32)
            nc.vector.tensor_mul(t1[:], u[:], u[:])
            nc.vector.tensor_scalar(t1[:], t1[:], -1.0, SCALE * SCALE, ALU.mult, ALU.add)
            nc.vector.tensor_scalar_max(t1[:], t1[:], 0.0)    # max(S^2 - u^2, 0)
            s_t = smal.tile([P, BT], dt.float32)
            nc.scalar.activation(s_t[:], t1[:], AF.Sqrt, scale=1.0)  # SCALE*sin_t
            t2 = smal.tile([P, BT], dt.float32)
            nc.vector.tensor_scalar_mul(t2[:], s_t[:], -sinm)
            t3 = smal.tile([P, BT], dt.float32)
            nc.vector.tensor_scalar_mul(t3[:], u[:], cosm - 1.0)
            delta = smal.tile([P, BT], dt.float32)
            nc.vector.tensor_add(delta[:], t2[:], t3[:])
            nc.sync.dma_start(dlt_e[:], delta[:])

        # ---------------- main class loop ------------------------------------
        def emit_mms(w_sb, cb, nb):
            pso = po_pool.tile([P, NF], dt.float32, tag="pso", name=f"pso{nb}")
            for k in range(KT):
                nc.tensor.matmul(
                    pso[:],
                    lhsT=w_sb[:, k, cb * P : (cb + 1) * P],
                    rhs=fT[:, k, nb * NF : (nb + 1) * NF],
                    start=(k == 0),
                    stop=(k == KT - 1),
                )
            return pso

        def emit_evac(pso, rnw, cb, nb, osb, cbi, eidx, act_major=True,
                      force_act=False):
            # og1 carries the next chunk's norm blocks, so tilt the
            # evacuation split toward DVE there (ACT holds the squares) —
            # EXCEPT its last c-blocks, whose banks the next og's first
            # matmuls recycle: those go ACT unconditionally so a DVE
            # backlog can't stall the PE across the og boundary
            if force_act or ((eidx % 5 < 3) if act_major else (eidx % 5 >= 2)):
                nc.scalar.activation(
                    osb[:, cbi, nb * NF : (nb + 1) * NF],
                    pso[:], AF.Copy,
                    scale=rnw[:, cb : cb + 1],
                )
            else:
                nc.vector.tensor_scalar_mul(
                    osb[:, cbi, nb * NF : (nb + 1) * NF],
                    pso[:], rnw[:, cb : cb + 1],
                )

        ready = {}   # ci -> (w_sb, rnw)
        raw = {}     # ci -> (w_sb, wn_sb)
        # chunk-0 norms: blocks 0-6 up front (og0's evacuations need them),
        # blocks 7-13 woven into the og0 nb1 pass so they can't queue on ACT
        # ahead of the nb1 evacuations; rsqrt in thirds
        rnw0 = smal.tile([P, CBK], dt.float32, tag="rnw", name="rnw0")
        nw20 = smal.tile([P, CBK], dt.float32, tag="nw2", name="nw20")
        for g in range(7):
            norm_block(wn_sb0, nw20, g)
            if g == 3:
                norm_rsqrt(nw20, rnw0, 0, 4)
        norm_rsqrt(nw20, rnw0, 4, 7)
        ready[0] = (w_sb0, rnw0)
        raw[1] = emit_load(1, rnf20[0:1, 6:7])
        osb_og0 = None  # og0 output tile of the current chunk (prefetch gate)

        for ci in range(CHUNKS):
            w_sb, rnw = ready.pop(ci)
            # norms for the NEXT chunk are spread over this chunk's two
            # output groups: ACT Square blocks (g 0-6) in og0's late slots,
            # DVE affine_mul_reduce blocks (g 7-13) in og1's early slots —
            # never a solid norm burst that starves evacuations and stalls
            # the PE on PSUM-bank recycling. Chunk 1's wn arrives mid-og1,
            # so its norms are emitted in bulk after chunk 0's og1 evacs.
            nxt = None
            if ci + 1 in raw and ci > 0:
                nw_sb, nwn_sb = raw.pop(ci + 1)
                nrnw = smal.tile([P, CBK], dt.float32, tag="rnw", name="rnw")
                nnw2 = smal.tile([P, CBK], dt.float32, tag="nw2", name="nw2")
                nxt = (nw_sb, nwn_sb, nrnw, nnw2)
            for og in range(CBK // OG):
                if og == 0 and ci == 2:
                    # margin input: gated on this chunk's rnw (~mid-kernel)
                    # so the greedy scheduler can't hoist the 1MB transfer
                    # into the startup window
                    gate(wt_sb[0:1, 0:1, 0:1], rnw[0:1, 0:1])
                    nc.gpsimd.dma_start(wt_sb[:], wtg_v)
                if og == 1 and ci == 0 and ci + 1 in raw:
                    nw_sb, nwn_sb = raw.pop(ci + 1)
                    nrnw = smal.tile([P, CBK], dt.float32, tag="rnw", name="rnw")
                    nnw2 = smal.tile([P, CBK], dt.float32, tag="nw2", name="nw2")
                    nxt = (nw_sb, nwn_sb, nrnw, nnw2)
                osb = opool.tile([P, OG, B], dt.bfloat16, tag="osb")
                if ci == 0 and og == 0:
                    # chunk-0 special: all nb=0 matmuls first (they only need
                    # the earlier fT tiles), then the nb=1 pass — the second
                    # batch half's transposes (emitted per-tile above) have
                    # ~12us of nb0 work to complete behind
                    for cbi in range(OG):
                        pso = emit_mms(w_sb, cbi, 0)
                        emit_evac(pso, rnw, cbi, 0, osb, cbi, cbi * NB)
                        if 3 <= cbi < 7:
                            emit_f_transpose(cbi + 1)
                    for cbi in range(OG):
                        pso = emit_mms(w_sb, cbi, 1)
                        emit_evac(pso, rnw, cbi, 1, osb, cbi, cbi * NB + 1)
                        norm_block(wn_sb0, nw20, 7 + cbi)
                    norm_rsqrt(nw20, rnw0, 7, CBK)
                else:
                    for cbi in range(OG):
                        cb = og * OG + cbi
                        pso_pair = [emit_mms(w_sb, cb, nb) for nb in range(NB)]
                        for nb in range(NB):
                            emit_evac(
                                pso_pair[nb], rnw, cb, nb, osb, cbi,
                                (ci * CBK + cb) * NB + nb,
                                act_major=(og == 0 or cbi >= 4),
                            )
                        if nxt is not None and ci > 0 and og == 1:
                            norm_block(nxt[1], nxt[3], 2 * cbi)
                            norm_block(nxt[1], nxt[3], 2 * cbi + 1)
                            if cbi == 3:
                                norm_rsqrt(nxt[3], nxt[2], 0, 7)
                        if og == 0 and ci == 3:
                            emit_margin_dot(cbi)
                        if og == 0 and ci == 4 and cbi == 0:
                            emit_margin_dot(7)
                        if og == 0 and ci == 4 and cbi == 1:
                            emit_margin()
                if og == 1 and nxt is not None:
                    if ci == 0:
                        # bulk after evacs (see above)
                        for g in range(CBK):
                            norm_block(nxt[1], nxt[3], g)
                            if g == 6:
                                norm_rsqrt(nxt[3], nxt[2], 0, 7)
                    norm_rsqrt(nxt[3], nxt[2], 7, CBK)
                    ready[ci + 1] = (nxt[0], nxt[2])
                if og == 0:
                    osb_og0 = osb
                if og == 1 and ci + 2 < CHUNKS:
                    raw[ci + 2] = emit_load(ci + 2, osb_og0[0:1, 0:1, 0:1])
                row0 = ci * CW + og * OG * P
                last_group = ci == CHUNKS - 1 and og == (CBK // OG) - 1
                if last_group:
                    # tail killer: ship the last group per-c-block across
                    # three rings so the kernel ends on a ~260KB transfer
                    engs = [nc.sync, nc.scalar, nc.gpsimd]
                    for cbi in range(OG):
                        engs[cbi % 3].dma_start(
                            out_e[row0 + cbi * P : row0 + (cbi + 1) * P, :]
                            .rearrange("(g p) b -> p g b", p=P),
                            osb[:, cbi : cbi + 1],
                        )
                else:
                    # one queue moves only ~25-34GB/s and a backed-up output
                    # stream delays osb recycling -> evacuations -> PSUM banks
                    # -> the PE itself; ship every group as three parallel
                    # pieces (scalar gets the biggest: it carries only f+
                    # outputs, while sync has wn and gpsimd has wT)
                    for eng, a, b in ((nc.scalar, 0, 3), (nc.sync, 3, 5),
                                      (nc.gpsimd, 5, 7)):
                        eng.dma_start(
                            out_e[row0 + a * P : row0 + b * P, :].rearrange(
                                "(g p) b -> p g b", p=P
                            ),
                            osb[:, a:b],
                        )

    nc.finalize()
    return nc


def _prep_inputs(features, targets, weights):
    import ml_dtypes

    f32 = np.ascontiguousarray(np.asarray(features, dtype=np.float32))
    tgt = np.asarray(targets).astype(np.int64)
    w = np.asarray(weights, dtype=np.float32)

    wide = (NCORES - 1) * CSTEP + CSH  # 100044
    wpad = np.zeros((wide, IN_F), dtype=np.float32)
    wpad[:OUT_C] = w
    wpad[OUT_C:, 0] = 1.0  # unit-norm filler rows: no inf/nan anywhere

    fb = f32.astype(ml_dtypes.bfloat16)
    in_maps = []
    for i in range(NCORES):
        sh = wpad[i * CSTEP : i * CSTEP + CSH]
        shb = sh.astype(ml_dtypes.bfloat16)
        wT = np.ascontiguousarray(shb.T)
        loc = np.clip(tgt - i * CSTEP, 0, CSH - 1)
        wtgt = np.ascontiguousarray(shb[loc])
        in_maps.append({"wT": wT, "wn": shb, "f": fb, "wtgt": wtgt})
    return in_maps, tgt


def kernel(features, targets, weights):
    global _GRAPH, LAST_EXEC_TIME_NS, LAST_RES
    from concourse.bass_utils import run_bass_kernel_spmd

    if _GRAPH is None:
        _GRAPH = _build_graph()
    nc = _GRAPH

    in_maps, tgt = _prep_inputs(features, targets, weights)

    trace = bool(int(os.environ.get("BASS_KERNEL_TRACE", "0")))
    res = run_bass_kernel_spmd(nc, in_maps, core_ids=list(range(NCORES)), trace=trace)
    LAST_EXEC_TIME_NS = res.exec_time_ns
    LAST_RES = res

    outs = [res.results[i]["out"][:CSTEP] for i in range(NCORES)]  # [12500, B]
    full = np.concatenate(outs, axis=0)                 # [100000, B] bf16
    logits = np.ascontiguousarray(full.T, dtype=np.float32)     # [B, OUT_C] f32

    # apply device-computed margin deltas at the 1024 target positions
    deltas = np.stack(
        [res.results[i]["delta"].T.reshape(B) for i in range(NCORES)]
    )  # [NCORES, B]; delta[p, t] -> b = t*128 + p
    rows = np.arange(B)
    core_of = np.minimum(tgt // CSTEP, NCORES - 1).astype(np.int64)
    logits[rows, tgt] += deltas[core_of, rows]
    return logits
